# revision 22
# baseline (speedup 1.0000x reference)
"""Chamfer distance kernel for Trainium2 (8 NeuronCores, SPMD).

Math: for point sets a[16384,3], b[16384,3],
  d2(i,j) = |a_i|^2 + |b_j|^2 - 2 a_i.b_j
encoded as an augmented inner product so the TensorEngine emits squared
distances directly.

fp32 matmuls on TRN2 are ~5x slower than bf16 (hi/lo double pass).  Each
fp32 operand is instead split into three bf16 pieces (value = h + m + l)
and the piece-products needed for ~fp32 accuracy are laid out along the
contraction axis: 8 piece-pairs per coordinate (only l*l dropped) = 24
rows, plus 3 rows for |b|^2 and 3 for |a|^2.  K=30 <= 32, so one bf16
matmul per tile computes exact-enough d2 (matmul cost scales with streamed
columns, not K).

The K<=32 contraction also enables 4-way row-group packing: operands are
replicated at SBUF partition offsets 0/32/64/96 and 4 matmuls run
concurrently in disjoint 32-row groups of the PE array via tile_position,
quadrupling TensorEngine throughput.

Reductions: every [128,2048] fp32 PSUM group must be min-reduced along the
free axis.  The DVE reduces fp32 PSUM at 1 elem/lane/cycle only; to beat
that, a fraction of groups is "ACT-assisted": ScalarE copies PSUM ->
SBUF bf16 (1x on the otherwise idle Scalar engine) and the DVE min-folds
bf16 tiles pairwise at its 2x packed rate.  The assist fraction is chosen
so ScalarE and VectorE finish together.

Sharding: a's rows split across 8 cores (2048 each); every core holds all
of b.  Each core computes row mins of its [2048, 16384] block (a->b) and,
via re-computing the block transposed, col mins (partial b->a).  sqrt and
the cross-core combine (elementwise min of 8 partial vectors + mean) run
on the host on 8*(2048+16384) floats; min/sqrt commute.
"""

import numpy as np

N = 16384          # points in each set
D = 3
NCORES = 8
NS = N // NCORES   # a-rows per core = 2048
K = 30             # split-precision contraction rows
KPAD = 32          # row-group stride for replicas
P = 128            # partitions
MM_N = 512         # matmul free dim per PSUM bank
GRP = 2048         # psum group = 4 matmuls of 512 (4 banks)

# column layout of the fused input tensor: [Wa shard | Rb | Wb | Ra shard]
OFF_WA = 0
OFF_RB = NS
OFF_WB = NS + N
OFF_RA = NS + N + N
TOT_COLS = 2 * (NS + N)

# ACT-assist schedules (tuned so ScalarE busy ~= VectorE busy).
# dir2 direct slots are placed so the assisted mm's between them come in
# contiguous runs of 4 (quad-folded with shared 3D-AP DVE ops).
D1_DIRECT = {0, 4}          # m-groups per n-chunk reduced directly from PSUM
D2_DIRECT = {0, 4, 8, 12}   # dir2: mm % 16 in this set -> direct reduce

_CACHE = {}


def _build_nc():
    from contextlib import ExitStack

    import concourse.bacc as bacc
    import concourse.mybir as mybir
    import concourse.tile as tile

    bf16 = mybir.dt.bfloat16
    f32 = mybir.dt.float32
    AX = mybir.AxisListType.X
    MIN = mybir.AluOpType.min

    nc = bacc.Bacc()
    aug = nc.dram_tensor("aug", [P, TOT_COLS], bf16, kind="ExternalInput")
    row_out = nc.dram_tensor("row_out", [P, NS // P], f32, kind="ExternalOutput")
    col_out = nc.dram_tensor("col_out", [P, N // P], f32, kind="ExternalOutput")

    with tile.TileContext(nc) as tc, ExitStack() as ctx:
        sb = ctx.enter_context(tc.tile_pool(name="sb", bufs=1))
        ps = ctx.enter_context(tc.tile_pool(name="ps", bufs=2, space="PSUM"))
        cnvp = ctx.enter_context(tc.tile_pool(name="cnvp", bufs=3))
        runp = ctx.enter_context(tc.tile_pool(name="runp", bufs=2))
        mn = ctx.enter_context(tc.tile_pool(name="mn", bufs=4))
        outp = ctx.enter_context(tc.tile_pool(name="outp", bufs=1))

        # Input DMA, parallelized across engines' HWDGE queues.  The first
        # matmul group needs only Wa + the first Rb group; that head slice is
        # split 4-ways by partition so it lands in ~1/4 the time.  The rest
        # streams in on parallel queues while compute runs.
        aug_sb = sb.tile([P, TOT_COLS], bf16)
        c1 = OFF_RB + GRP
        qengines = [nc.sync, nc.scalar, nc.sync, nc.scalar]
        for qi, eng in enumerate(qengines):
            eng.dma_start(
                out=aug_sb[qi * 32:(qi + 1) * 32, 0:c1],
                in_=aug[qi * 32:(qi + 1) * 32, 0:c1],
            )
        rest = [(c1, OFF_RB + N // 2), (OFF_RB + N // 2, OFF_WB),
                (OFF_WB, OFF_WB + N // 2), (OFF_WB + N // 2, TOT_COLS)]
        for qi, (lo, hi) in enumerate(rest):
            qengines[qi].dma_start(out=aug_sb[:, lo:hi], in_=aug[:, lo:hi])

        row_acc = outp.tile([P, NS // P], f32)
        col_acc = outp.tile([P, N // P], f32)

        def packed_group(pt, w_off, r_off):
            """4 concurrent matmuls (row groups g=0..3) filling pt[128,2048].
            Row group g handles the g-th 512-column sub-slice."""
            for g in range(4):
                bp = KPAD * g
                nc.tensor.matmul(
                    pt[:, g * MM_N:(g + 1) * MM_N],
                    aug_sb[bp:bp + K, w_off:w_off + P],
                    aug_sb[bp:bp + K, r_off + g * MM_N:r_off + (g + 1) * MM_N],
                    start=True,
                    stop=True,
                    tile_position=(bp, 0),
                )

        def fold_to_col(src_bf, dst_col):
            """src_bf [128,2048] bf16 -> min over free axis -> dst_col [128,1]."""
            f1 = runp.tile([P, 1024], bf16, tag="f1")
            nc.vector.tensor_tensor(
                out=f1[:, :], in0=src_bf[:, 0:1024], in1=src_bf[:, 1024:2048], op=MIN
            )
            f2 = runp.tile([P, 512], bf16, tag="f2")
            nc.vector.tensor_tensor(
                out=f2[:, :], in0=f1[:, 0:512], in1=f1[:, 512:1024], op=MIN
            )
            nc.vector.tensor_reduce(dst_col, f2[:, :], axis=AX, op=MIN)

        # ---- direction 1: a-shard rows on partitions, min over all b
        n_chunks = NS // P              # 16
        m_groups = N // GRP             # 8
        for n in range(n_chunks):
            d1_direct = D1_DIRECT if n % 2 == 0 else {mg for mg in D1_DIRECT if mg == 0}
            n_direct = len(d1_direct)
            minlets = mn.tile([P, n_direct + 1], f32, tag="minlets")
            run = None
            di = 0
            for mg in range(m_groups):
                pt = ps.tile([P, GRP], f32, tag="pt")
                packed_group(pt, OFF_WA + n * P, OFF_RB + mg * GRP)
                if mg in d1_direct:
                    nc.vector.tensor_reduce(
                        minlets[:, di:di + 1], pt[:, :], axis=AX, op=MIN
                    )
                    di += 1
                else:
                    cnv = cnvp.tile([P, GRP], bf16, tag="cnv")
                    nc.scalar.copy(cnv[:, :], pt[:, :])
                    if run is None:
                        run = cnv
                    else:
                        nrun = runp.tile([P, GRP], bf16, tag="run")
                        nc.vector.tensor_tensor(
                            out=nrun[:, :], in0=run[:, :], in1=cnv[:, :], op=MIN
                        )
                        run = nrun
            fold_to_col(run, minlets[:, n_direct:n_direct + 1])
            nc.vector.tensor_reduce(
                row_acc[:, n:n + 1], minlets[:, :], axis=AX, op=MIN
            )
            if n % 8 == 7:
                nc.sync.dma_start(
                    out=row_out[:, n - 7:n + 1], in_=row_acc[:, n - 7:n + 1]
                )

        # ---- direction 2: b rows on partitions, min over this a-shard
        def flush_quad(quad, mm0, cnt):
            """Fold `cnt` (<=4) converted groups packed in quad[128, cnt*2048]
            down to col_acc[:, mm0:mm0+cnt] with shared 3D-AP DVE ops."""
            cur = quad.rearrange("p (g f) -> p g f", g=4)[:, 0:cnt, :]
            width = GRP
            while width > 128:
                half = width // 2
                nxt = runp.tile([P, 4, half], bf16, tag=f"q{half}")
                nc.vector.tensor_tensor(
                    out=nxt[:, 0:cnt, :],
                    in0=cur[:, :, 0:half],
                    in1=cur[:, :, half:width],
                    op=MIN,
                )
                cur = nxt[:, 0:cnt, :]
                width = half
            nc.vector.tensor_reduce(
                col_acc[:, mm0:mm0 + cnt], cur, axis=AX, op=MIN
            )

        mm_chunks = N // P              # 128
        quad = None
        qcnt = 0
        qstart = 0
        for mm in range(mm_chunks):
            pt = ps.tile([P, GRP], f32, tag="pt")
            packed_group(pt, OFF_WB + mm * P, OFF_RA)
            if (mm % 16) in D2_DIRECT:
                if qcnt:
                    flush_quad(quad, qstart, qcnt)
                    quad, qcnt = None, 0
                nc.vector.tensor_reduce(
                    col_acc[:, mm:mm + 1], pt[:, :], axis=AX, op=MIN
                )
            else:
                if quad is None:
                    quad = cnvp.tile([P, 4 * GRP], bf16, tag="quad")
                    qstart = mm
                nc.scalar.copy(quad[:, qcnt * GRP:(qcnt + 1) * GRP], pt[:, :])
                qcnt += 1
                if qcnt == 4:
                    flush_quad(quad, qstart, 4)
                    quad, qcnt = None, 0
            if mm % 32 == 31:
                if qcnt:
                    flush_quad(quad, qstart, qcnt)
                    quad, qcnt = None, 0
                nc.sync.dma_start(
                    out=col_out[:, mm - 31:mm + 1], in_=col_acc[:, mm - 31:mm + 1]
                )
        if qcnt:
            flush_quad(quad, qstart, qcnt)

    nc.compile()
    return nc


def _get_nc():
    if "nc" not in _CACHE:
        _CACHE["nc"] = _build_nc()
    return _CACHE["nc"]


def _install_ntff_hook():
    """The agent image's `antenv` lacks `axon_hooks`; provide it so
    run_bass_kernel_spmd(trace=True) can profile via the axon PJRT .so."""
    import sys

    if "antenv.axon_hooks" in sys.modules:
        return
    try:
        import contextlib
        import ctypes
        import types

        so_path = "/opt/axon/libaxon_pjrt.so"
        lib = ctypes.CDLL(so_path)
        if not hasattr(lib, "axon_start_nrt_profile"):
            return
        lib.axon_start_nrt_profile.argtypes = [
            ctypes.POINTER(ctypes.c_int64),
            ctypes.c_size_t,
        ]
        lib.axon_start_nrt_profile.restype = ctypes.c_int64
        lib.axon_stop_nrt_profile.argtypes = [ctypes.c_char_p]
        lib.axon_stop_nrt_profile.restype = ctypes.c_int64

        @contextlib.contextmanager
        def _hook(output_dir, device_ids):
            import jax

            jax.devices()
            if device_ids:
                ids = (ctypes.c_int64 * len(device_ids))(*device_ids)
                rc = lib.axon_start_nrt_profile(ids, len(device_ids))
            else:
                rc = lib.axon_start_nrt_profile(None, 0)
            if rc != 0:
                raise RuntimeError(f"axon_start_nrt_profile rc={rc}")
            try:
                yield
            finally:
                n = lib.axon_stop_nrt_profile(str(output_dir).encode())
                if n < 0:
                    raise RuntimeError(f"axon_stop_nrt_profile rc={n}")

        mod = types.ModuleType("antenv.axon_hooks")
        mod.get_axon_ntff_profile_hook = lambda: _hook
        mod.set_axon_ntff_profile_hook = lambda h: None
        sys.modules["antenv.axon_hooks"] = mod
    except Exception:
        pass


def _run(in_maps, trace=False):
    from concourse.bass_utils import run_bass_kernel_spmd

    if trace:
        _install_ntff_hook()
    nc = _get_nc()
    res = run_bass_kernel_spmd(
        nc, in_maps, core_ids=list(range(NCORES)), trace=trace
    )
    _CACHE["last_exec_ns"] = res.exec_time_ns
    _CACHE["last_trace"] = res.instructions_and_trace
    return res.results


def _split3(x):
    """fp32 -> three bf16 pieces (returned as fp32 for further math)."""
    import ml_dtypes

    h = x.astype(ml_dtypes.bfloat16).astype(np.float32)
    r = x - h
    m = r.astype(ml_dtypes.bfloat16).astype(np.float32)
    l = (r - m).astype(np.float32)
    return h, m, l


# piece-pair schedule per coordinate: indices into (h, m, l)
_PAIRS = [(0, 0), (0, 1), (1, 0), (0, 2), (2, 0), (1, 1), (1, 2), (2, 1)]


def _build_wr(Pts, Qts, P2, Q2):
    """W from the stationary set (with -2*coords and |P|^2), R from the
    streaming set (coords and |Q|^2), such that W[:, i] . R[:, j] = d2."""
    W = np.zeros((K, Pts.shape[0]), np.float32)
    R = np.zeros((K, Qts.shape[0]), np.float32)
    k = 0
    for d in range(D):
        u = _split3(-2.0 * Pts[:, d])
        v = _split3(Qts[:, d])
        for wp, rp in _PAIRS:
            W[k] = u[wp]
            R[k] = v[rp]
            k += 1
    q2p = _split3(Q2)
    for t in range(3):
        W[k] = 1.0
        R[k] = q2p[t]
        k += 1
    p2p = _split3(P2)
    for t in range(3):
        W[k] = p2p[t]
        R[k] = 1.0
        k += 1
    assert k == K
    return W, R


def kernel(a, b):
    import ml_dtypes
    import os

    a = np.ascontiguousarray(np.asarray(a, dtype=np.float32))
    b = np.ascontiguousarray(np.asarray(b, dtype=np.float32))
    assert a.shape == (N, D) and b.shape == (N, D), (a.shape, b.shape)

    a2 = np.sum(a.astype(np.float64) * a, axis=1).astype(np.float32)
    b2 = np.sum(b.astype(np.float64) * b, axis=1).astype(np.float32)

    Wa, Rb = _build_wr(a, b, a2, b2)   # direction 1: a stationary, b streaming
    Wb, Ra = _build_wr(b, a, b2, a2)   # direction 2: b stationary, a streaming

    trace = bool(int(os.environ.get("CHAMFER_TRACE", "0")))
    in_maps = []
    for r in range(NCORES):
        row = np.zeros((KPAD, TOT_COLS), np.float32)
        row[:K, OFF_WA:OFF_WA + NS] = Wa[:, r * NS:(r + 1) * NS]
        row[:K, OFF_RB:OFF_RB + N] = Rb
        row[:K, OFF_WB:OFF_WB + N] = Wb
        row[:K, OFF_RA:OFF_RA + NS] = Ra[:, r * NS:(r + 1) * NS]
        buf = np.tile(row, (4, 1))          # replicas at partitions 0/32/64/96
        in_maps.append({"aug": buf.astype(ml_dtypes.bfloat16)})
    results = _run(in_maps, trace=trace)

    # row_out[p, n] -> row index i = n*128 + p ; shards in core order
    rows = np.concatenate(
        [results[r]["row_out"].T.reshape(-1) for r in range(NCORES)]
    )
    # col partials: min over cores
    cols = np.min(
        np.stack([results[r]["col_out"].T.reshape(-1) for r in range(NCORES)]),
        axis=0,
    )
    mins_sq = np.concatenate([rows, cols])
    dist = np.sqrt(np.maximum(mins_sq, 0.0))
    return np.asarray(np.mean(dist), dtype=np.float32)


# revision 23
# speedup vs baseline: 1.1349x; 1.1349x over previous
"""Chamfer distance kernel for Trainium2 (8 NeuronCores, SPMD).

Math: for point sets a[16384,3], b[16384,3],
  d2(i,j) = |a_i|^2 + |b_j|^2 - 2 a_i.b_j
encoded as an augmented inner product so the TensorEngine emits (negated)
squared distances directly; every reduction is then a MAX of -d2 (the
GPSIMD partition reduce only supports max, and min/max are symmetric).

fp32 matmuls on TRN2 are ~5x slower than bf16 (hi/lo double pass).  Each
fp32 operand is instead split into three bf16 pieces (value = h + m + l)
and the piece-products needed for ~fp32 accuracy are laid out along the
contraction axis (only l*l dropped): 24 coordinate rows + 3 |b|^2 rows +
3 |a|^2 rows = K=30 <= 32, so ONE bf16 matmul per tile computes -d2 at
fp32-grade accuracy (matmul cost scales with streamed columns, not K).

K<=32 also enables 4-way row-group packing: operands are replicated at
SBUF partition offsets 0/32/64/96 and 4 matmuls run concurrently in
disjoint 32-row groups of the PE array via tile_position.

Dataflow per core (a-rows sharded, 2048 per core; b replicated):
  PE    : -d2 psum groups [128, 2048] fp32      (a-chunk x b-group)
  ACT   : copy psum -> SBUF bf16 (ScalarE is the only other engine that
          can read PSUM; DVE fp32-PSUM reads are capped at 1 elem/cycle)
  DVE   : per group, TWO bf16 tensor_tensor max ops at the 2x packed rate:
            run_row[n]  = max(run_row[n],  t)   (a->b direction)
            run_col[mg] = max(run_col[mg], t)   (b->a direction, partial)
  DVE   : fold run_row[n] along free axis -> per-a-point max
  GPSIMD: partition_all_reduce(max) folds run_col across partitions
          (the only engine that can reduce the partition axis; it is
          otherwise idle)
Loop order is m-group outer / a-chunk inner so each run_col finalizes
early and its partition reduce overlaps the next group's stream.

Host: negate, sqrt, combine the 8 cores' partial b->a vectors with an
elementwise min, mean.  (min/sqrt commute; host work is 8*18k floats.)
"""

import numpy as np

N = 16384          # points in each set
D = 3
NCORES = 8
NS = N // NCORES   # a-rows per core = 2048
K = 30             # split-precision contraction rows
KPAD = 32          # row-group stride for replicas
P = 128            # partitions
MM_N = 512         # matmul free dim per PSUM bank
GRP = 2048         # psum group = 4 matmuls of 512 (4 banks)

# column layout of the fused input tensor: [Wa shard | Rb]
OFF_WA = 0
OFF_RB = NS
TOT_COLS = NS + N

NEG_INF = -3.0e38

_CACHE = {}


def _build_nc():
    from contextlib import ExitStack

    import concourse.bacc as bacc
    import concourse.bass_isa as bass_isa
    import concourse.mybir as mybir
    import concourse.tile as tile

    bf16 = mybir.dt.bfloat16
    f32 = mybir.dt.float32
    AX = mybir.AxisListType.X
    MAX = mybir.AluOpType.max

    nc = bacc.Bacc()
    aug = nc.dram_tensor("aug", [P, TOT_COLS], bf16, kind="ExternalInput")
    # row_out[p, n] = max_j -d2(a[n*128+p], b[j])
    # col_out[mg, c] = max over this core's a of -d2(a_i, b[mg*2048+c])
    row_out = nc.dram_tensor("row_out", [P, NS // P], f32, kind="ExternalOutput")
    col_out = nc.dram_tensor("col_out", [N // GRP, GRP], f32, kind="ExternalOutput")

    n_chunks = NS // P              # 16
    m_groups = N // GRP             # 8

    with tile.TileContext(nc) as tc, ExitStack() as ctx:
        sb = ctx.enter_context(tc.tile_pool(name="sb", bufs=1))
        ps = ctx.enter_context(tc.tile_pool(name="ps", bufs=2, space="PSUM"))
        cnvp = ctx.enter_context(tc.tile_pool(name="cnvp", bufs=4))
        runp = ctx.enter_context(tc.tile_pool(name="runp", bufs=2))
        colp = ctx.enter_context(tc.tile_pool(name="colp", bufs=2))
        outp = ctx.enter_context(tc.tile_pool(name="outp", bufs=1))

        # Input DMA parallelized across the two HWDGE-capable engines; the
        # head slice (Wa + first Rb group) is partition-split so the first
        # matmul can start in ~1/4 the time.
        aug_sb = sb.tile([P, TOT_COLS], bf16)
        c1 = OFF_RB + GRP
        qengines = [nc.sync, nc.scalar, nc.sync, nc.scalar]
        for qi, eng in enumerate(qengines):
            eng.dma_start(
                out=aug_sb[qi * 32:(qi + 1) * 32, 0:c1],
                in_=aug[qi * 32:(qi + 1) * 32, 0:c1],
            )
        half = OFF_RB + GRP + (TOT_COLS - c1) // 2
        nc.sync.dma_start(out=aug_sb[:, c1:half], in_=aug[:, c1:half])
        nc.scalar.dma_start(out=aug_sb[:, half:], in_=aug[:, half:])

        # Per-a-chunk running row maxes, alive across the whole kernel.
        run_rows = sb.tile([P, n_chunks, GRP], bf16)
        nc.gpsimd.memset(run_rows[:, :, :], NEG_INF)

        row_acc = outp.tile([P, NS // P], f32)

        def packed_group(pt, w_off, r_off):
            """4 concurrent matmuls (row groups g=0..3) filling pt[128,2048].
            Row group g handles the g-th 512-column sub-slice."""
            for g in range(4):
                bp = KPAD * g
                nc.tensor.matmul(
                    pt[:, g * MM_N:(g + 1) * MM_N],
                    aug_sb[bp:bp + K, w_off:w_off + P],
                    aug_sb[bp:bp + K, r_off + g * MM_N:r_off + (g + 1) * MM_N],
                    start=True,
                    stop=True,
                    tile_position=(bp, 0),
                )

        def fold_row(n):
            """run_rows[:, n, :] -> max over free axis -> row_acc[:, n]."""
            f1 = runp.tile([P, 1024], bf16, tag="f1")
            nc.vector.tensor_tensor(
                out=f1[:, :], in0=run_rows[:, n, 0:1024],
                in1=run_rows[:, n, 1024:2048], op=MAX,
            )
            f2 = runp.tile([P, 512], bf16, tag="f2")
            nc.vector.tensor_tensor(
                out=f2[:, :], in0=f1[:, 0:512], in1=f1[:, 512:1024], op=MAX,
            )
            nc.vector.tensor_reduce(row_acc[:, n:n + 1], f2[:, :], axis=AX, op=MAX)

        for mg in range(m_groups):
            run_col = colp.tile([P, GRP], bf16, tag="run_col")
            nc.gpsimd.memset(run_col[:, :], NEG_INF)
            for n in range(n_chunks):
                pt = ps.tile([P, GRP], f32, tag="pt")
                packed_group(pt, OFF_WA + n * P, OFF_RB + mg * GRP)
                t = cnvp.tile([P, GRP], bf16, tag="cnv")
                nc.scalar.copy(t[:, :], pt[:, :])
                nc.vector.tensor_tensor(
                    out=run_rows[:, n, :], in0=run_rows[:, n, :], in1=t[:, :],
                    op=MAX,
                )
                nc.vector.tensor_tensor(
                    out=run_col[:, :], in0=run_col[:, :], in1=t[:, :], op=MAX,
                )
                if mg == m_groups - 1:
                    fold_row(n)
            pr = colp.tile([P, GRP], f32, tag="pr")
            nc.gpsimd.partition_all_reduce(
                pr[:, :], run_col[:, :], channels=P,
                reduce_op=bass_isa.ReduceOp.max,
            )
            nc.sync.dma_start(out=col_out[mg:mg + 1, :], in_=pr[0:1, :])
        nc.sync.dma_start(out=row_out[:, :], in_=row_acc[:, :])

    nc.compile()
    return nc


def _get_nc():
    if "nc" not in _CACHE:
        _CACHE["nc"] = _build_nc()
    return _CACHE["nc"]


def _install_ntff_hook():
    """The agent image's `antenv` lacks `axon_hooks`; provide it so
    run_bass_kernel_spmd(trace=True) can profile via the axon PJRT .so."""
    import sys

    if "antenv.axon_hooks" in sys.modules:
        return
    try:
        import contextlib
        import ctypes
        import types

        so_path = "/opt/axon/libaxon_pjrt.so"
        lib = ctypes.CDLL(so_path)
        if not hasattr(lib, "axon_start_nrt_profile"):
            return
        lib.axon_start_nrt_profile.argtypes = [
            ctypes.POINTER(ctypes.c_int64),
            ctypes.c_size_t,
        ]
        lib.axon_start_nrt_profile.restype = ctypes.c_int64
        lib.axon_stop_nrt_profile.argtypes = [ctypes.c_char_p]
        lib.axon_stop_nrt_profile.restype = ctypes.c_int64

        @contextlib.contextmanager
        def _hook(output_dir, device_ids):
            import jax

            jax.devices()
            if device_ids:
                ids = (ctypes.c_int64 * len(device_ids))(*device_ids)
                rc = lib.axon_start_nrt_profile(ids, len(device_ids))
            else:
                rc = lib.axon_start_nrt_profile(None, 0)
            if rc != 0:
                raise RuntimeError(f"axon_start_nrt_profile rc={rc}")
            try:
                yield
            finally:
                n = lib.axon_stop_nrt_profile(str(output_dir).encode())
                if n < 0:
                    raise RuntimeError(f"axon_stop_nrt_profile rc={n}")

        mod = types.ModuleType("antenv.axon_hooks")
        mod.get_axon_ntff_profile_hook = lambda: _hook
        mod.set_axon_ntff_profile_hook = lambda h: None
        sys.modules["antenv.axon_hooks"] = mod
    except Exception:
        pass


def _run(in_maps, trace=False):
    from concourse.bass_utils import run_bass_kernel_spmd

    if trace:
        _install_ntff_hook()
    nc = _get_nc()
    res = run_bass_kernel_spmd(
        nc, in_maps, core_ids=list(range(NCORES)), trace=trace
    )
    _CACHE["last_exec_ns"] = res.exec_time_ns
    _CACHE["last_trace"] = res.instructions_and_trace
    return res.results


def _split3(x):
    """fp32 -> three bf16 pieces (returned as fp32 for further math)."""
    import ml_dtypes

    h = x.astype(ml_dtypes.bfloat16).astype(np.float32)
    r = x - h
    m = r.astype(ml_dtypes.bfloat16).astype(np.float32)
    l = (r - m).astype(np.float32)
    return h, m, l


# piece-pair schedule per coordinate: indices into (h, m, l)
_PAIRS = [(0, 0), (0, 1), (1, 0), (0, 2), (2, 0), (1, 1), (1, 2), (2, 1)]


def _build_wr(Pts, Qts, P2, Q2):
    """W from the stationary set, R from the streaming set, such that
    W[:, i] . R[:, j] = -d2(P_i, Q_j)  (negated for max-reductions)."""
    W = np.zeros((K, Pts.shape[0]), np.float32)
    R = np.zeros((K, Qts.shape[0]), np.float32)
    k = 0
    for d in range(D):
        u = _split3(2.0 * Pts[:, d])       # +2 a_d  (negated -2 a.b term)
        v = _split3(Qts[:, d])
        for wp, rp in _PAIRS:
            W[k] = u[wp]
            R[k] = v[rp]
            k += 1
    q2p = _split3(Q2)
    for t in range(3):
        W[k] = -1.0
        R[k] = q2p[t]
        k += 1
    p2p = _split3(P2)
    for t in range(3):
        W[k] = -p2p[t]
        R[k] = 1.0
        k += 1
    assert k == K
    return W, R


def kernel(a, b):
    import ml_dtypes
    import os

    a = np.ascontiguousarray(np.asarray(a, dtype=np.float32))
    b = np.ascontiguousarray(np.asarray(b, dtype=np.float32))
    assert a.shape == (N, D) and b.shape == (N, D), (a.shape, b.shape)

    a2 = np.sum(a.astype(np.float64) * a, axis=1).astype(np.float32)
    b2 = np.sum(b.astype(np.float64) * b, axis=1).astype(np.float32)

    Wa, Rb = _build_wr(a, b, a2, b2)

    trace = bool(int(os.environ.get("CHAMFER_TRACE", "0")))
    in_maps = []
    for r in range(NCORES):
        row = np.zeros((KPAD, TOT_COLS), np.float32)
        row[:K, OFF_WA:OFF_WA + NS] = Wa[:, r * NS:(r + 1) * NS]
        row[:K, OFF_RB:OFF_RB + N] = Rb
        buf = np.tile(row, (4, 1))          # replicas at partitions 0/32/64/96
        in_maps.append({"aug": buf.astype(ml_dtypes.bfloat16)})
    results = _run(in_maps, trace=trace)

    # row_out[p, n] -> row index i = n*128 + p ; shards in core order
    rows = np.concatenate(
        [-results[r]["row_out"].T.reshape(-1) for r in range(NCORES)]
    )
    # col partials (negated maxes): global min = -max over cores
    cols = -np.max(
        np.stack([results[r]["col_out"].reshape(-1) for r in range(NCORES)]),
        axis=0,
    )
    mins_sq = np.concatenate([rows, cols])
    dist = np.sqrt(np.maximum(mins_sq, 0.0))
    return np.asarray(np.mean(dist), dtype=np.float32)


# revision 26
# speedup vs baseline: 1.2458x; 1.0977x over previous
"""Chamfer distance kernel for Trainium2 (8 NeuronCores, SPMD).

Math: for point sets a[16384,3], b[16384,3],
  d2(i,j) = |a_i|^2 + |b_j|^2 - 2 a_i.b_j
encoded as an augmented inner product so the TensorEngine emits (negated)
squared distances directly; every reduction is then a MAX of -d2 (the
GPSIMD partition reduce only supports max, and min/max are symmetric).

fp32 matmuls on TRN2 are ~5x slower than bf16 (hi/lo double pass).  Each
fp32 operand is instead split into three bf16 pieces (value = h + m + l)
and the piece-products needed for ~fp32 accuracy are laid out along the
contraction axis (only l*l dropped): 24 coordinate rows + 3 |b|^2 rows +
3 |a|^2 rows = K=30 <= 32, so ONE bf16 matmul per tile computes -d2 at
fp32-grade accuracy (matmul cost scales with streamed columns, not K).

K<=32 also enables 4-way row-group packing: operands are replicated at
SBUF partition offsets 0/32/64/96 and 4 matmuls run concurrently in
disjoint 32-row groups of the PE array via tile_position.

Dataflow per core (a-rows sharded, 2048 per core; b replicated):
  PE    : -d2 psum groups [128, 2048] fp32      (a-chunk x b-group)
  ACT   : copy psum -> SBUF bf16 (ScalarE is the only other engine that
          can read PSUM; DVE fp32-PSUM reads are capped at 1 elem/cycle)
  DVE   : per group, TWO bf16 tensor_tensor max ops at the 2x packed rate:
            run_row[n]  = max(run_row[n],  t)   (a->b direction)
            run_col[mg] = max(run_col[mg], t)   (b->a direction, partial)
  DVE   : fold run_row[n] along free axis -> per-a-point max
  GPSIMD: partition_all_reduce(max) folds run_col across partitions
          (the only engine that can reduce the partition axis; it is
          otherwise idle)
Loop order is m-group outer / a-chunk inner so each run_col finalizes
early and its partition reduce overlaps the next group's stream.

Host: negate, sqrt, combine the 8 cores' partial b->a vectors with an
elementwise min, mean.  (min/sqrt commute; host work is 8*18k floats.)
"""

import numpy as np

N = 16384          # points in each set
D = 3
NCORES = 8
NS = N // NCORES   # a-rows per core = 2048
K = 30             # split-precision contraction rows
KPAD = 32          # row-group stride for replicas
P = 128            # partitions
MM_N = 512         # matmul free dim per PSUM bank
GRP = 2048         # psum group = 4 matmuls of 512 (4 banks)

# column layout of the fused input tensor: [Wa shard | Rb]
OFF_WA = 0
OFF_RB = NS
TOT_COLS = NS + N

NEG_INF = -3.0e38

_CACHE = {}


def _build_nc():
    from contextlib import ExitStack

    import concourse.bacc as bacc
    import concourse.bass_isa as bass_isa
    import concourse.mybir as mybir
    import concourse.tile as tile

    bf16 = mybir.dt.bfloat16
    f32 = mybir.dt.float32
    AX = mybir.AxisListType.X
    MAX = mybir.AluOpType.max

    nc = bacc.Bacc()
    aug = nc.dram_tensor("aug", [P, TOT_COLS], bf16, kind="ExternalInput")
    # row_out[p, n] = max_j -d2(a[n*128+p], b[j])
    # col_out[mg, c] = max over this core's a of -d2(a_i, b[mg*2048+c])
    row_out = nc.dram_tensor("row_out", [P, NS // P], f32, kind="ExternalOutput")
    col_out = nc.dram_tensor("col_out", [N // GRP, GRP], f32, kind="ExternalOutput")

    n_chunks = NS // P              # 16
    m_groups = N // GRP             # 8

    with tile.TileContext(nc) as tc, ExitStack() as ctx:
        sb = ctx.enter_context(tc.tile_pool(name="sb", bufs=1))
        ps = ctx.enter_context(tc.tile_pool(name="ps", bufs=2, space="PSUM"))
        cnvp = ctx.enter_context(tc.tile_pool(name="cnvp", bufs=4))
        runp = ctx.enter_context(tc.tile_pool(name="runp", bufs=2))
        colp = ctx.enter_context(tc.tile_pool(name="colp", bufs=3))
        outp = ctx.enter_context(tc.tile_pool(name="outp", bufs=1))

        # Input DMA parallelized across the two HWDGE-capable engines; the
        # head slice (Wa + first Rb group) is partition-split so the first
        # matmul can start in ~1/4 the time.
        aug_sb = sb.tile([P, TOT_COLS], bf16)
        c1 = OFF_RB + GRP
        qengines = [nc.sync, nc.scalar, nc.sync, nc.scalar]
        for qi, eng in enumerate(qengines):
            eng.dma_start(
                out=aug_sb[qi * 32:(qi + 1) * 32, 0:c1],
                in_=aug[qi * 32:(qi + 1) * 32, 0:c1],
            )
        half = OFF_RB + GRP + (TOT_COLS - c1) // 2
        nc.sync.dma_start(out=aug_sb[:, c1:half], in_=aug[:, c1:half])
        nc.scalar.dma_start(out=aug_sb[:, half:], in_=aug[:, half:])

        # Per-a-chunk running row maxes, alive across the whole kernel.
        # Initialized by copying the first m-group's tile (no memset needed).
        run_rows = sb.tile([P, n_chunks, GRP], bf16)

        row_acc = outp.tile([P, NS // P], f32)

        def packed_group(pt, w_off, r_off):
            """4 concurrent matmuls (row groups g=0..3) filling pt[128,2048].
            Row group g handles the g-th 512-column sub-slice."""
            for g in range(4):
                bp = KPAD * g
                nc.tensor.matmul(
                    pt[:, g * MM_N:(g + 1) * MM_N],
                    aug_sb[bp:bp + K, w_off:w_off + P],
                    aug_sb[bp:bp + K, r_off + g * MM_N:r_off + (g + 1) * MM_N],
                    start=True,
                    stop=True,
                    tile_position=(bp, 0),
                )

        def fold_row(n):
            """run_rows[:, n, :] -> max over free axis -> row_acc[:, n]."""
            f1 = runp.tile([P, 1024], bf16, tag="f1")
            nc.vector.tensor_tensor(
                out=f1[:, :], in0=run_rows[:, n, 0:1024],
                in1=run_rows[:, n, 1024:2048], op=MAX,
            )
            f2 = runp.tile([P, 512], bf16, tag="f2")
            nc.vector.tensor_tensor(
                out=f2[:, :], in0=f1[:, 0:512], in1=f1[:, 512:1024], op=MAX,
            )
            nc.vector.tensor_reduce(row_acc[:, n:n + 1], f2[:, :], axis=AX, op=MAX)

        for mg in range(m_groups):
            run_col = colp.tile([P, GRP], bf16, tag="run_col")
            for n in range(n_chunks):
                pt = ps.tile([P, GRP], f32, tag="pt")
                packed_group(pt, OFF_WA + n * P, OFF_RB + mg * GRP)
                t = cnvp.tile([P, GRP], bf16, tag="cnv")
                nc.scalar.copy(t[:, :], pt[:, :])
                if mg == 0:
                    nc.vector.tensor_copy(run_rows[:, n, :], t[:, :])
                else:
                    nc.vector.tensor_tensor(
                        out=run_rows[:, n, :], in0=run_rows[:, n, :],
                        in1=t[:, :], op=MAX,
                    )
                if n == 0:
                    nc.vector.tensor_copy(run_col[:, :], t[:, :])
                else:
                    nc.vector.tensor_tensor(
                        out=run_col[:, :], in0=run_col[:, :], in1=t[:, :], op=MAX,
                    )
                if mg == m_groups - 1:
                    fold_row(n)
            pr = colp.tile([P, GRP], f32, tag="pr")
            nc.gpsimd.partition_all_reduce(
                pr[:, :], run_col[:, :], channels=P,
                reduce_op=bass_isa.ReduceOp.max,
            )
            nc.sync.dma_start(out=col_out[mg:mg + 1, :], in_=pr[0:1, :])
        nc.sync.dma_start(out=row_out[:, :], in_=row_acc[:, :])

    nc.compile()
    return nc


def _get_nc():
    if "nc" not in _CACHE:
        _CACHE["nc"] = _build_nc()
    return _CACHE["nc"]


def _install_ntff_hook():
    """The agent image's `antenv` lacks `axon_hooks`; provide it so
    run_bass_kernel_spmd(trace=True) can profile via the axon PJRT .so."""
    import sys

    if "antenv.axon_hooks" in sys.modules:
        return
    try:
        import contextlib
        import ctypes
        import types

        so_path = "/opt/axon/libaxon_pjrt.so"
        lib = ctypes.CDLL(so_path)
        if not hasattr(lib, "axon_start_nrt_profile"):
            return
        lib.axon_start_nrt_profile.argtypes = [
            ctypes.POINTER(ctypes.c_int64),
            ctypes.c_size_t,
        ]
        lib.axon_start_nrt_profile.restype = ctypes.c_int64
        lib.axon_stop_nrt_profile.argtypes = [ctypes.c_char_p]
        lib.axon_stop_nrt_profile.restype = ctypes.c_int64

        @contextlib.contextmanager
        def _hook(output_dir, device_ids):
            import jax

            jax.devices()
            if device_ids:
                ids = (ctypes.c_int64 * len(device_ids))(*device_ids)
                rc = lib.axon_start_nrt_profile(ids, len(device_ids))
            else:
                rc = lib.axon_start_nrt_profile(None, 0)
            if rc != 0:
                raise RuntimeError(f"axon_start_nrt_profile rc={rc}")
            try:
                yield
            finally:
                n = lib.axon_stop_nrt_profile(str(output_dir).encode())
                if n < 0:
                    raise RuntimeError(f"axon_stop_nrt_profile rc={n}")

        mod = types.ModuleType("antenv.axon_hooks")
        mod.get_axon_ntff_profile_hook = lambda: _hook
        mod.set_axon_ntff_profile_hook = lambda h: None
        sys.modules["antenv.axon_hooks"] = mod
    except Exception:
        pass


def _run(in_maps, trace=False):
    from concourse.bass_utils import run_bass_kernel_spmd

    if trace:
        _install_ntff_hook()
    nc = _get_nc()
    res = run_bass_kernel_spmd(
        nc, in_maps, core_ids=list(range(NCORES)), trace=trace
    )
    _CACHE["last_exec_ns"] = res.exec_time_ns
    _CACHE["last_trace"] = res.instructions_and_trace
    return res.results


def _split3(x):
    """fp32 -> three bf16 pieces (returned as fp32 for further math)."""
    import ml_dtypes

    h = x.astype(ml_dtypes.bfloat16).astype(np.float32)
    r = x - h
    m = r.astype(ml_dtypes.bfloat16).astype(np.float32)
    l = (r - m).astype(np.float32)
    return h, m, l


# piece-pair schedule per coordinate: indices into (h, m, l)
_PAIRS = [(0, 0), (0, 1), (1, 0), (0, 2), (2, 0), (1, 1), (1, 2), (2, 1)]


def _build_wr(Pts, Qts, P2, Q2):
    """W from the stationary set, R from the streaming set, such that
    W[:, i] . R[:, j] = -d2(P_i, Q_j)  (negated for max-reductions)."""
    W = np.zeros((K, Pts.shape[0]), np.float32)
    R = np.zeros((K, Qts.shape[0]), np.float32)
    k = 0
    for d in range(D):
        u = _split3(2.0 * Pts[:, d])       # +2 a_d  (negated -2 a.b term)
        v = _split3(Qts[:, d])
        for wp, rp in _PAIRS:
            W[k] = u[wp]
            R[k] = v[rp]
            k += 1
    q2p = _split3(Q2)
    for t in range(3):
        W[k] = -1.0
        R[k] = q2p[t]
        k += 1
    p2p = _split3(P2)
    for t in range(3):
        W[k] = -p2p[t]
        R[k] = 1.0
        k += 1
    assert k == K
    return W, R


def kernel(a, b):
    import ml_dtypes
    import os

    a = np.ascontiguousarray(np.asarray(a, dtype=np.float32))
    b = np.ascontiguousarray(np.asarray(b, dtype=np.float32))
    assert a.shape == (N, D) and b.shape == (N, D), (a.shape, b.shape)

    a2 = np.sum(a.astype(np.float64) * a, axis=1).astype(np.float32)
    b2 = np.sum(b.astype(np.float64) * b, axis=1).astype(np.float32)

    Wa, Rb = _build_wr(a, b, a2, b2)

    trace = bool(int(os.environ.get("CHAMFER_TRACE", "0")))
    in_maps = []
    for r in range(NCORES):
        row = np.zeros((KPAD, TOT_COLS), np.float32)
        row[:K, OFF_WA:OFF_WA + NS] = Wa[:, r * NS:(r + 1) * NS]
        row[:K, OFF_RB:OFF_RB + N] = Rb
        buf = np.tile(row, (4, 1))          # replicas at partitions 0/32/64/96
        in_maps.append({"aug": buf.astype(ml_dtypes.bfloat16)})
    results = _run(in_maps, trace=trace)

    # row_out[p, n] -> row index i = n*128 + p ; shards in core order
    rows = np.concatenate(
        [-results[r]["row_out"].T.reshape(-1) for r in range(NCORES)]
    )
    # col partials (negated maxes): global min = -max over cores
    cols = -np.max(
        np.stack([results[r]["col_out"].reshape(-1) for r in range(NCORES)]),
        axis=0,
    )
    mins_sq = np.concatenate([rows, cols])
    dist = np.sqrt(np.maximum(mins_sq, 0.0))
    return np.asarray(np.mean(dist), dtype=np.float32)


# revision 31
# speedup vs baseline: 1.2640x; 1.0146x over previous
"""Chamfer distance kernel for Trainium2 (8 NeuronCores, SPMD).

Math: for point sets a[16384,3], b[16384,3],
  d2(i,j) = |a_i|^2 + |b_j|^2 - 2 a_i.b_j
encoded as an augmented inner product so the TensorEngine emits (negated)
squared distances directly; every reduction is then a MAX of -d2 (the
GPSIMD partition reduce only supports max, and min/max are symmetric).

fp32 matmuls on TRN2 are ~5x slower than bf16 (hi/lo double pass).  Each
fp32 operand is instead split into three bf16 pieces (value = h + m + l)
and the piece-products needed for ~fp32 accuracy are laid out along the
contraction axis (only l*l dropped): 24 coordinate rows + 3 |b|^2 rows +
3 |a|^2 rows = K=30 <= 32, so ONE bf16 matmul per tile computes -d2 at
fp32-grade accuracy (matmul cost scales with streamed columns, not K).

K<=32 also enables 4-way row-group packing: operands are replicated at
SBUF partition offsets 0/32/64/96 and 4 matmuls run concurrently in
disjoint 32-row groups of the PE array via tile_position.

Dataflow per core (a-rows sharded, 2048 per core; b replicated):
  PE    : -d2 psum groups [128, 2048] fp32      (a-chunk x b-group)
  ACT   : copy psum -> SBUF bf16 (ScalarE is the only other engine that
          can read PSUM; DVE fp32-PSUM reads are capped at 1 elem/cycle)
  DVE   : per group, TWO bf16 tensor_tensor max ops at the 2x packed rate:
            run_row[n]  = max(run_row[n],  t)   (a->b direction)
            run_col[mg] = max(run_col[mg], t)   (b->a direction, partial)
  DVE   : fold run_row[n] along free axis -> per-a-point max
  GPSIMD: partition_all_reduce(max) folds run_col across partitions
          (the only engine that can reduce the partition axis; it is
          otherwise idle)
Loop order is m-group outer / a-chunk inner so each run_col finalizes
early and its partition reduce overlaps the next group's stream.

Host: negate, sqrt, combine the 8 cores' partial b->a vectors with an
elementwise min, mean.  (min/sqrt commute; host work is 8*18k floats.)
"""

import numpy as np

N = 16384          # points in each set
D = 3
NCORES = 8
NS = N // NCORES   # a-rows per core = 2048
K = 30             # split-precision contraction rows
KPAD = 32          # row-group stride for replicas
P = 128            # partitions
MM_N = 512         # matmul free dim per PSUM bank
GRP = 2048         # psum group = 4 matmuls of 512 (4 banks)

# column layout of the fused input tensor: [Wa shard | Rb]
OFF_WA = 0
OFF_RB = NS
TOT_COLS = NS + N

NEG_INF = -3.0e38

_CACHE = {}


def _build_nc():
    from contextlib import ExitStack

    import concourse.bacc as bacc
    import concourse.bass_isa as bass_isa
    import concourse.mybir as mybir
    import concourse.tile as tile

    bf16 = mybir.dt.bfloat16
    f32 = mybir.dt.float32
    AX = mybir.AxisListType.X
    MAX = mybir.AluOpType.max

    nc = bacc.Bacc()
    aug = nc.dram_tensor("aug", [P, TOT_COLS], bf16, kind="ExternalInput")
    # row_out[p, n] = max_j -d2(a[n*128+p], b[j])
    # col_out[mg, c] = max over this core's a of -d2(a_i, b[mg*2048+c])
    # (the last m-group is reduced via PE transposes instead of the GPSIMD
    # partition reduce so it doesn't trail the kernel; its layout is
    # col7_out[p, t] = col max for j = 7*2048 + t*128 + p)
    row_out = nc.dram_tensor("row_out", [P, NS // P], f32, kind="ExternalOutput")
    col_out = nc.dram_tensor(
        "col_out", [N // GRP - 1, GRP], f32, kind="ExternalOutput"
    )
    col7_out = nc.dram_tensor("col7_out", [P, GRP // P], f32, kind="ExternalOutput")

    n_chunks = NS // P              # 16
    m_groups = N // GRP             # 8

    with tile.TileContext(nc) as tc, ExitStack() as ctx:
        sb = ctx.enter_context(tc.tile_pool(name="sb", bufs=1))
        ps = ctx.enter_context(tc.tile_pool(name="ps", bufs=2, space="PSUM"))
        cnvp = ctx.enter_context(tc.tile_pool(name="cnvp", bufs=4))
        runp = ctx.enter_context(tc.tile_pool(name="runp", bufs=2))
        colp = ctx.enter_context(tc.tile_pool(name="colp", bufs=4))
        outp = ctx.enter_context(tc.tile_pool(name="outp", bufs=1))

        # Input DMA parallelized across the two HWDGE-capable engines; the
        # head slice (Wa + first Rb group) is partition-split so the first
        # matmul can start in ~1/4 the time.
        aug_sb = sb.tile([P, TOT_COLS], bf16)
        c1 = OFF_RB + GRP
        qengines = [nc.sync, nc.scalar, nc.sync, nc.scalar]
        for qi, eng in enumerate(qengines):
            eng.dma_start(
                out=aug_sb[qi * 32:(qi + 1) * 32, 0:c1],
                in_=aug[qi * 32:(qi + 1) * 32, 0:c1],
            )
        half = OFF_RB + GRP + (TOT_COLS - c1) // 2
        nc.sync.dma_start(out=aug_sb[:, c1:half], in_=aug[:, c1:half])
        nc.scalar.dma_start(out=aug_sb[:, half:], in_=aug[:, half:])

        # Per-a-chunk running row maxes, alive across the whole kernel.
        # Initialized by copying the first m-group's tile (no memset needed).
        run_rows = sb.tile([P, n_chunks, GRP], bf16)

        row_acc = outp.tile([P, NS // P], f32)
        col7_acc = outp.tile([P, GRP // P], f32)

        from concourse.masks import make_identity

        ident = sb.tile([P, P], bf16)
        make_identity(nc, ident[:, :])

        def packed_group(pt, w_off, r_off):
            """4 concurrent matmuls (row groups g=0..3) filling pt[128,2048].
            Row group g handles the g-th 512-column sub-slice."""
            for g in range(4):
                bp = KPAD * g
                nc.tensor.matmul(
                    pt[:, g * MM_N:(g + 1) * MM_N],
                    aug_sb[bp:bp + K, w_off:w_off + P],
                    aug_sb[bp:bp + K, r_off + g * MM_N:r_off + (g + 1) * MM_N],
                    start=True,
                    stop=True,
                    tile_position=(bp, 0),
                )

        def fold_row(n):
            """run_rows[:, n, :] -> max over free axis -> row_acc[:, n]."""
            f1 = runp.tile([P, 1024], bf16, tag="f1")
            nc.vector.tensor_tensor(
                out=f1[:, :], in0=run_rows[:, n, 0:1024],
                in1=run_rows[:, n, 1024:2048], op=MAX,
            )
            f2 = runp.tile([P, 512], bf16, tag="f2")
            nc.vector.tensor_tensor(
                out=f2[:, :], in0=f1[:, 0:512], in1=f1[:, 512:1024], op=MAX,
            )
            nc.vector.tensor_reduce(row_acc[:, n:n + 1], f2[:, :], axis=AX, op=MAX)

        for mg in range(m_groups):
            run_col = colp.tile([P, GRP], bf16, tag="run_col")
            for n in range(n_chunks):
                pt = ps.tile([P, GRP], f32, tag="pt")
                packed_group(pt, OFF_WA + n * P, OFF_RB + mg * GRP)
                t = cnvp.tile([P, GRP], bf16, tag="cnv")
                nc.scalar.copy(t[:, :], pt[:, :])
                if mg == 0:
                    nc.vector.tensor_copy(run_rows[:, n, :], t[:, :])
                else:
                    nc.vector.tensor_tensor(
                        out=run_rows[:, n, :], in0=run_rows[:, n, :],
                        in1=t[:, :], op=MAX,
                    )
                if n == 0:
                    nc.vector.tensor_copy(run_col[:, :], t[:, :])
                else:
                    nc.vector.tensor_tensor(
                        out=run_col[:, :], in0=run_col[:, :], in1=t[:, :], op=MAX,
                    )
                if mg == m_groups - 1:
                    fold_row(n)
            if mg < m_groups - 1:
                pr = colp.tile([P, GRP], f32, tag="pr")
                nc.gpsimd.partition_all_reduce(
                    pr[:, :], run_col[:, :], channels=P,
                    reduce_op=bass_isa.ReduceOp.max,
                )
                nc.sync.dma_start(out=col_out[mg:mg + 1, :], in_=pr[0:1, :])
            else:
                # Tail m-group: partition-reduce via PE transposes + DVE
                # (PE/DVE are idle by now; GPSIMD would trail the kernel).
                for tb in range(GRP // P):
                    tp = ps.tile([P, P], bf16, tag="pt")
                    nc.tensor.transpose(
                        tp[:, :], run_col[:, tb * P:(tb + 1) * P], ident[:, :]
                    )
                    nc.vector.tensor_reduce(
                        col7_acc[:, tb:tb + 1], tp[:, :], axis=AX, op=MAX
                    )
                nc.sync.dma_start(out=col7_out[:, :], in_=col7_acc[:, :])
        nc.sync.dma_start(out=row_out[:, :], in_=row_acc[:, :])

    nc.compile()
    return nc


def _get_nc():
    if "nc" not in _CACHE:
        _CACHE["nc"] = _build_nc()
    return _CACHE["nc"]


def _install_ntff_hook():
    """The agent image's `antenv` lacks `axon_hooks`; provide it so
    run_bass_kernel_spmd(trace=True) can profile via the axon PJRT .so."""
    import sys

    if "antenv.axon_hooks" in sys.modules:
        return
    try:
        import contextlib
        import ctypes
        import types

        so_path = "/opt/axon/libaxon_pjrt.so"
        lib = ctypes.CDLL(so_path)
        if not hasattr(lib, "axon_start_nrt_profile"):
            return
        lib.axon_start_nrt_profile.argtypes = [
            ctypes.POINTER(ctypes.c_int64),
            ctypes.c_size_t,
        ]
        lib.axon_start_nrt_profile.restype = ctypes.c_int64
        lib.axon_stop_nrt_profile.argtypes = [ctypes.c_char_p]
        lib.axon_stop_nrt_profile.restype = ctypes.c_int64

        @contextlib.contextmanager
        def _hook(output_dir, device_ids):
            import jax

            jax.devices()
            if device_ids:
                ids = (ctypes.c_int64 * len(device_ids))(*device_ids)
                rc = lib.axon_start_nrt_profile(ids, len(device_ids))
            else:
                rc = lib.axon_start_nrt_profile(None, 0)
            if rc != 0:
                raise RuntimeError(f"axon_start_nrt_profile rc={rc}")
            try:
                yield
            finally:
                n = lib.axon_stop_nrt_profile(str(output_dir).encode())
                if n < 0:
                    raise RuntimeError(f"axon_stop_nrt_profile rc={n}")

        mod = types.ModuleType("antenv.axon_hooks")
        mod.get_axon_ntff_profile_hook = lambda: _hook
        mod.set_axon_ntff_profile_hook = lambda h: None
        sys.modules["antenv.axon_hooks"] = mod
    except Exception:
        pass


def _run(in_maps, trace=False):
    from concourse.bass_utils import run_bass_kernel_spmd

    if trace:
        _install_ntff_hook()
    nc = _get_nc()
    res = run_bass_kernel_spmd(
        nc, in_maps, core_ids=list(range(NCORES)), trace=trace
    )
    _CACHE["last_exec_ns"] = res.exec_time_ns
    _CACHE["last_trace"] = res.instructions_and_trace
    return res.results


def _split3(x):
    """fp32 -> three bf16 pieces (returned as fp32 for further math)."""
    import ml_dtypes

    h = x.astype(ml_dtypes.bfloat16).astype(np.float32)
    r = x - h
    m = r.astype(ml_dtypes.bfloat16).astype(np.float32)
    l = (r - m).astype(np.float32)
    return h, m, l


# piece-pair schedule per coordinate: indices into (h, m, l)
_PAIRS = [(0, 0), (0, 1), (1, 0), (0, 2), (2, 0), (1, 1), (1, 2), (2, 1)]


def _build_wr(Pts, Qts, P2, Q2):
    """W from the stationary set, R from the streaming set, such that
    W[:, i] . R[:, j] = -d2(P_i, Q_j)  (negated for max-reductions)."""
    W = np.zeros((K, Pts.shape[0]), np.float32)
    R = np.zeros((K, Qts.shape[0]), np.float32)
    k = 0
    for d in range(D):
        u = _split3(2.0 * Pts[:, d])       # +2 a_d  (negated -2 a.b term)
        v = _split3(Qts[:, d])
        for wp, rp in _PAIRS:
            W[k] = u[wp]
            R[k] = v[rp]
            k += 1
    q2p = _split3(Q2)
    for t in range(3):
        W[k] = -1.0
        R[k] = q2p[t]
        k += 1
    p2p = _split3(P2)
    for t in range(3):
        W[k] = -p2p[t]
        R[k] = 1.0
        k += 1
    assert k == K
    return W, R


def kernel(a, b):
    import ml_dtypes
    import os

    a = np.ascontiguousarray(np.asarray(a, dtype=np.float32))
    b = np.ascontiguousarray(np.asarray(b, dtype=np.float32))
    assert a.shape == (N, D) and b.shape == (N, D), (a.shape, b.shape)

    a2 = np.sum(a.astype(np.float64) * a, axis=1).astype(np.float32)
    b2 = np.sum(b.astype(np.float64) * b, axis=1).astype(np.float32)

    Wa, Rb = _build_wr(a, b, a2, b2)

    trace = bool(int(os.environ.get("CHAMFER_TRACE", "0")))
    in_maps = []
    for r in range(NCORES):
        row = np.zeros((KPAD, TOT_COLS), np.float32)
        row[:K, OFF_WA:OFF_WA + NS] = Wa[:, r * NS:(r + 1) * NS]
        row[:K, OFF_RB:OFF_RB + N] = Rb
        buf = np.tile(row, (4, 1))          # replicas at partitions 0/32/64/96
        in_maps.append({"aug": buf.astype(ml_dtypes.bfloat16)})
    results = _run(in_maps, trace=trace)

    # row_out[p, n] -> row index i = n*128 + p ; shards in core order
    rows = np.concatenate(
        [-results[r]["row_out"].T.reshape(-1) for r in range(NCORES)]
    )
    # col partials (negated maxes): global min = -max over cores.
    # first 7 m-groups from col_out [7,2048]; last from col7_out [128,16]
    # where j = 7*2048 + t*128 + p.
    def core_cols(r):
        c = np.empty(N, np.float32)
        c[0:7 * GRP] = results[r]["col_out"].reshape(-1)
        c[7 * GRP:] = results[r]["col7_out"].T.reshape(-1)
        return c

    cols = -np.max(np.stack([core_cols(r) for r in range(NCORES)]), axis=0)
    mins_sq = np.concatenate([rows, cols])
    dist = np.sqrt(np.maximum(mins_sq, 0.0))
    return np.asarray(np.mean(dist), dtype=np.float32)


# revision 32
# speedup vs baseline: 1.2651x; 1.0009x over previous
"""Chamfer distance kernel for Trainium2 (8 NeuronCores, SPMD).

Math: for point sets a[16384,3], b[16384,3],
  d2(i,j) = |a_i|^2 + |b_j|^2 - 2 a_i.b_j
encoded as an augmented inner product so the TensorEngine emits (negated)
squared distances directly; every reduction is then a MAX of -d2 (the
GPSIMD partition reduce only supports max, and min/max are symmetric).

fp32 matmuls on TRN2 are ~5x slower than bf16 (hi/lo double pass).  Each
fp32 operand is instead split into three bf16 pieces (value = h + m + l)
and the piece-products needed for ~fp32 accuracy are laid out along the
contraction axis (only l*l dropped): 24 coordinate rows + 3 |b|^2 rows +
3 |a|^2 rows = K=30 <= 32, so ONE bf16 matmul per tile computes -d2 at
fp32-grade accuracy (matmul cost scales with streamed columns, not K).

K<=32 also enables 4-way row-group packing: operands are replicated at
SBUF partition offsets 0/32/64/96 and 4 matmuls run concurrently in
disjoint 32-row groups of the PE array via tile_position.

Dataflow per core (a-rows sharded, 2048 per core; b replicated):
  PE    : -d2 psum groups [128, 2048] fp32      (a-chunk x b-group)
  ACT   : copy psum -> SBUF bf16 (ScalarE is the only other engine that
          can read PSUM; DVE fp32-PSUM reads are capped at 1 elem/cycle)
  DVE   : per group, TWO bf16 tensor_tensor max ops at the 2x packed rate:
            run_row[n]  = max(run_row[n],  t)   (a->b direction)
            run_col[mg] = max(run_col[mg], t)   (b->a direction, partial)
  DVE   : fold run_row[n] along free axis -> per-a-point max
  GPSIMD: partition_all_reduce(max) folds run_col across partitions
          (the only engine that can reduce the partition axis; it is
          otherwise idle)
Loop order is m-group outer / a-chunk inner so each run_col finalizes
early and its partition reduce overlaps the next group's stream.

Host: negate, sqrt, combine the 8 cores' partial b->a vectors with an
elementwise min, mean.  (min/sqrt commute; host work is 8*18k floats.)
"""

import numpy as np

N = 16384          # points in each set
D = 3
NCORES = 8
NS = N // NCORES   # a-rows per core = 2048
K = 30             # split-precision contraction rows
KPAD = 32          # row-group stride for replicas
P = 128            # partitions
MM_N = 512         # matmul free dim per PSUM bank
GRP = 2048         # psum group = 4 matmuls of 512 (4 banks)

# column layout of the fused input tensor: [Wa shard | Rb]
OFF_WA = 0
OFF_RB = NS
TOT_COLS = NS + N

NEG_INF = -3.0e38

_CACHE = {}


def _build_nc():
    from contextlib import ExitStack

    import concourse.bacc as bacc
    import concourse.bass_isa as bass_isa
    import concourse.mybir as mybir
    import concourse.tile as tile

    bf16 = mybir.dt.bfloat16
    f32 = mybir.dt.float32
    AX = mybir.AxisListType.X
    MAX = mybir.AluOpType.max

    nc = bacc.Bacc()
    aug = nc.dram_tensor("aug", [P, TOT_COLS], bf16, kind="ExternalInput")
    # row_out[p, n] = max_j -d2(a[n*128+p], b[j])
    # col_out[mg, c] = max over this core's a of -d2(a_i, b[mg*2048+c])
    # (the last m-group is reduced via PE transposes instead of the GPSIMD
    # partition reduce so it doesn't trail the kernel; its layout is
    # col7_out[p, t] = col max for j = 7*2048 + t*128 + p)
    row_out = nc.dram_tensor("row_out", [P, NS // P], f32, kind="ExternalOutput")
    col_out = nc.dram_tensor(
        "col_out", [N // GRP - 1, GRP], f32, kind="ExternalOutput"
    )
    col7_out = nc.dram_tensor("col7_out", [P, GRP // P], f32, kind="ExternalOutput")

    n_chunks = NS // P              # 16
    m_groups = N // GRP             # 8

    with tile.TileContext(nc) as tc, ExitStack() as ctx:
        sb = ctx.enter_context(tc.tile_pool(name="sb", bufs=1))
        ps = ctx.enter_context(tc.tile_pool(name="ps", bufs=2, space="PSUM"))
        cnvp = ctx.enter_context(tc.tile_pool(name="cnvp", bufs=4))
        runp = ctx.enter_context(tc.tile_pool(name="runp", bufs=2))
        colp = ctx.enter_context(tc.tile_pool(name="colp", bufs=4))
        outp = ctx.enter_context(tc.tile_pool(name="outp", bufs=1))

        # Input DMA parallelized across the two HWDGE-capable engines; the
        # head slice (Wa + first Rb group) is partition-split so the first
        # matmul can start in ~1/4 the time.
        aug_sb = sb.tile([P, TOT_COLS], bf16)
        c1 = OFF_RB + GRP
        qengines = [nc.sync, nc.scalar, nc.sync, nc.scalar]
        for qi, eng in enumerate(qengines):
            eng.dma_start(
                out=aug_sb[qi * 32:(qi + 1) * 32, 0:c1],
                in_=aug[qi * 32:(qi + 1) * 32, 0:c1],
            )
        # bulk input rides the scalar-engine HWDGE queue (measured much
        # faster than the sync queue, which also carries the outputs)
        half = OFF_RB + GRP + (TOT_COLS - c1) // 2
        nc.scalar.dma_start(out=aug_sb[:, c1:half], in_=aug[:, c1:half])
        nc.scalar.dma_start(out=aug_sb[:, half:], in_=aug[:, half:])

        # Per-a-chunk running row maxes, alive across the whole kernel.
        # Initialized by copying the first m-group's tile (no memset needed).
        run_rows = sb.tile([P, n_chunks, GRP], bf16)

        row_acc = outp.tile([P, NS // P], f32)
        col7_acc = outp.tile([P, GRP // P], f32)

        from concourse.masks import make_identity

        ident = sb.tile([P, P], bf16)
        make_identity(nc, ident[:, :])

        def packed_group(pt, w_off, r_off):
            """4 concurrent matmuls (row groups g=0..3) filling pt[128,2048].
            Row group g handles the g-th 512-column sub-slice."""
            for g in range(4):
                bp = KPAD * g
                nc.tensor.matmul(
                    pt[:, g * MM_N:(g + 1) * MM_N],
                    aug_sb[bp:bp + K, w_off:w_off + P],
                    aug_sb[bp:bp + K, r_off + g * MM_N:r_off + (g + 1) * MM_N],
                    start=True,
                    stop=True,
                    tile_position=(bp, 0),
                )

        def fold_row(n):
            """run_rows[:, n, :] -> max over free axis -> row_acc[:, n]."""
            f1 = runp.tile([P, 1024], bf16, tag="f1")
            nc.vector.tensor_tensor(
                out=f1[:, :], in0=run_rows[:, n, 0:1024],
                in1=run_rows[:, n, 1024:2048], op=MAX,
            )
            f2 = runp.tile([P, 512], bf16, tag="f2")
            nc.vector.tensor_tensor(
                out=f2[:, :], in0=f1[:, 0:512], in1=f1[:, 512:1024], op=MAX,
            )
            nc.vector.tensor_reduce(row_acc[:, n:n + 1], f2[:, :], axis=AX, op=MAX)

        for mg in range(m_groups):
            run_col = colp.tile([P, GRP], bf16, tag="run_col")
            for n in range(n_chunks):
                pt = ps.tile([P, GRP], f32, tag="pt")
                packed_group(pt, OFF_WA + n * P, OFF_RB + mg * GRP)
                t = cnvp.tile([P, GRP], bf16, tag="cnv")
                nc.scalar.copy(t[:, :], pt[:, :])
                if mg == 0:
                    nc.vector.tensor_copy(run_rows[:, n, :], t[:, :])
                else:
                    nc.vector.tensor_tensor(
                        out=run_rows[:, n, :], in0=run_rows[:, n, :],
                        in1=t[:, :], op=MAX,
                    )
                if n == 0:
                    nc.vector.tensor_copy(run_col[:, :], t[:, :])
                else:
                    nc.vector.tensor_tensor(
                        out=run_col[:, :], in0=run_col[:, :], in1=t[:, :], op=MAX,
                    )
                if mg == m_groups - 1:
                    fold_row(n)
            if mg < m_groups - 1:
                pr = colp.tile([P, GRP], f32, tag="pr")
                nc.gpsimd.partition_all_reduce(
                    pr[:, :], run_col[:, :], channels=P,
                    reduce_op=bass_isa.ReduceOp.max,
                )
                nc.sync.dma_start(out=col_out[mg:mg + 1, :], in_=pr[0:1, :])
            else:
                # Tail m-group: partition-reduce via PE transposes + DVE
                # (PE/DVE are idle by now; GPSIMD would trail the kernel).
                for tb in range(GRP // P):
                    tp = ps.tile([P, P], bf16, tag="pt")
                    nc.tensor.transpose(
                        tp[:, :], run_col[:, tb * P:(tb + 1) * P], ident[:, :]
                    )
                    nc.vector.tensor_reduce(
                        col7_acc[:, tb:tb + 1], tp[:, :], axis=AX, op=MAX
                    )
                nc.sync.dma_start(out=col7_out[:, :], in_=col7_acc[:, :])
        nc.sync.dma_start(out=row_out[:, :], in_=row_acc[:, :])

    nc.compile()
    return nc


def _get_nc():
    if "nc" not in _CACHE:
        _CACHE["nc"] = _build_nc()
    return _CACHE["nc"]


def _install_ntff_hook():
    """The agent image's `antenv` lacks `axon_hooks`; provide it so
    run_bass_kernel_spmd(trace=True) can profile via the axon PJRT .so."""
    import sys

    if "antenv.axon_hooks" in sys.modules:
        return
    try:
        import contextlib
        import ctypes
        import types

        so_path = "/opt/axon/libaxon_pjrt.so"
        lib = ctypes.CDLL(so_path)
        if not hasattr(lib, "axon_start_nrt_profile"):
            return
        lib.axon_start_nrt_profile.argtypes = [
            ctypes.POINTER(ctypes.c_int64),
            ctypes.c_size_t,
        ]
        lib.axon_start_nrt_profile.restype = ctypes.c_int64
        lib.axon_stop_nrt_profile.argtypes = [ctypes.c_char_p]
        lib.axon_stop_nrt_profile.restype = ctypes.c_int64

        @contextlib.contextmanager
        def _hook(output_dir, device_ids):
            import jax

            jax.devices()
            if device_ids:
                ids = (ctypes.c_int64 * len(device_ids))(*device_ids)
                rc = lib.axon_start_nrt_profile(ids, len(device_ids))
            else:
                rc = lib.axon_start_nrt_profile(None, 0)
            if rc != 0:
                raise RuntimeError(f"axon_start_nrt_profile rc={rc}")
            try:
                yield
            finally:
                n = lib.axon_stop_nrt_profile(str(output_dir).encode())
                if n < 0:
                    raise RuntimeError(f"axon_stop_nrt_profile rc={n}")

        mod = types.ModuleType("antenv.axon_hooks")
        mod.get_axon_ntff_profile_hook = lambda: _hook
        mod.set_axon_ntff_profile_hook = lambda h: None
        sys.modules["antenv.axon_hooks"] = mod
    except Exception:
        pass


def _run(in_maps, trace=False):
    from concourse.bass_utils import run_bass_kernel_spmd

    if trace:
        _install_ntff_hook()
    nc = _get_nc()
    res = run_bass_kernel_spmd(
        nc, in_maps, core_ids=list(range(NCORES)), trace=trace
    )
    _CACHE["last_exec_ns"] = res.exec_time_ns
    _CACHE["last_trace"] = res.instructions_and_trace
    return res.results


def _split3(x):
    """fp32 -> three bf16 pieces (returned as fp32 for further math)."""
    import ml_dtypes

    h = x.astype(ml_dtypes.bfloat16).astype(np.float32)
    r = x - h
    m = r.astype(ml_dtypes.bfloat16).astype(np.float32)
    l = (r - m).astype(np.float32)
    return h, m, l


# piece-pair schedule per coordinate: indices into (h, m, l)
_PAIRS = [(0, 0), (0, 1), (1, 0), (0, 2), (2, 0), (1, 1), (1, 2), (2, 1)]


def _build_wr(Pts, Qts, P2, Q2):
    """W from the stationary set, R from the streaming set, such that
    W[:, i] . R[:, j] = -d2(P_i, Q_j)  (negated for max-reductions)."""
    W = np.zeros((K, Pts.shape[0]), np.float32)
    R = np.zeros((K, Qts.shape[0]), np.float32)
    k = 0
    for d in range(D):
        u = _split3(2.0 * Pts[:, d])       # +2 a_d  (negated -2 a.b term)
        v = _split3(Qts[:, d])
        for wp, rp in _PAIRS:
            W[k] = u[wp]
            R[k] = v[rp]
            k += 1
    q2p = _split3(Q2)
    for t in range(3):
        W[k] = -1.0
        R[k] = q2p[t]
        k += 1
    p2p = _split3(P2)
    for t in range(3):
        W[k] = -p2p[t]
        R[k] = 1.0
        k += 1
    assert k == K
    return W, R


def kernel(a, b):
    import ml_dtypes
    import os

    a = np.ascontiguousarray(np.asarray(a, dtype=np.float32))
    b = np.ascontiguousarray(np.asarray(b, dtype=np.float32))
    assert a.shape == (N, D) and b.shape == (N, D), (a.shape, b.shape)

    a2 = np.sum(a.astype(np.float64) * a, axis=1).astype(np.float32)
    b2 = np.sum(b.astype(np.float64) * b, axis=1).astype(np.float32)

    Wa, Rb = _build_wr(a, b, a2, b2)

    trace = bool(int(os.environ.get("CHAMFER_TRACE", "0")))
    in_maps = []
    for r in range(NCORES):
        row = np.zeros((KPAD, TOT_COLS), np.float32)
        row[:K, OFF_WA:OFF_WA + NS] = Wa[:, r * NS:(r + 1) * NS]
        row[:K, OFF_RB:OFF_RB + N] = Rb
        buf = np.tile(row, (4, 1))          # replicas at partitions 0/32/64/96
        in_maps.append({"aug": buf.astype(ml_dtypes.bfloat16)})
    results = _run(in_maps, trace=trace)

    # row_out[p, n] -> row index i = n*128 + p ; shards in core order
    rows = np.concatenate(
        [-results[r]["row_out"].T.reshape(-1) for r in range(NCORES)]
    )
    # col partials (negated maxes): global min = -max over cores.
    # first 7 m-groups from col_out [7,2048]; last from col7_out [128,16]
    # where j = 7*2048 + t*128 + p.
    def core_cols(r):
        c = np.empty(N, np.float32)
        c[0:7 * GRP] = results[r]["col_out"].reshape(-1)
        c[7 * GRP:] = results[r]["col7_out"].T.reshape(-1)
        return c

    cols = -np.max(np.stack([core_cols(r) for r in range(NCORES)]), axis=0)
    mins_sq = np.concatenate([rows, cols])
    dist = np.sqrt(np.maximum(mins_sq, 0.0))
    return np.asarray(np.mean(dist), dtype=np.float32)


# revision 34
# speedup vs baseline: 1.2662x; 1.0009x over previous
"""Chamfer distance kernel for Trainium2 (8 NeuronCores, SPMD).

Math: for point sets a[16384,3], b[16384,3],
  d2(i,j) = |a_i|^2 + |b_j|^2 - 2 a_i.b_j
encoded as an augmented inner product so the TensorEngine emits (negated)
squared distances directly; every reduction is then a MAX of -d2 (the
GPSIMD partition reduce only supports max, and min/max are symmetric).

fp32 matmuls on TRN2 are ~5x slower than bf16 (hi/lo double pass).  Each
fp32 operand is instead split into three bf16 pieces (value = h + m + l)
and the piece-products needed for ~fp32 accuracy are laid out along the
contraction axis (only l*l dropped): 24 coordinate rows + 3 |b|^2 rows +
3 |a|^2 rows = K=30 <= 32, so ONE bf16 matmul per tile computes -d2 at
fp32-grade accuracy (matmul cost scales with streamed columns, not K).

K<=32 also enables 4-way row-group packing: operands are replicated at
SBUF partition offsets 0/32/64/96 and 4 matmuls run concurrently in
disjoint 32-row groups of the PE array via tile_position.

Dataflow per core (a-rows sharded, 2048 per core; b replicated):
  PE    : -d2 psum groups [128, 2048] fp32      (a-chunk x b-group)
  ACT   : copy psum -> SBUF bf16 (ScalarE is the only other engine that
          can read PSUM; DVE fp32-PSUM reads are capped at 1 elem/cycle)
  DVE   : per group, TWO bf16 tensor_tensor max ops at the 2x packed rate:
            run_row[n]  = max(run_row[n],  t)   (a->b direction)
            run_col[mg] = max(run_col[mg], t)   (b->a direction, partial)
  DVE   : fold run_row[n] along free axis -> per-a-point max
  GPSIMD: partition_all_reduce(max) folds run_col across partitions
          (the only engine that can reduce the partition axis; it is
          otherwise idle)
Loop order is m-group outer / a-chunk inner so each run_col finalizes
early and its partition reduce overlaps the next group's stream.

Host: negate, sqrt, combine the 8 cores' partial b->a vectors with an
elementwise min, mean.  (min/sqrt commute; host work is 8*18k floats.)
"""

import numpy as np

N = 16384          # points in each set
D = 3
NCORES = 8
NS = N // NCORES   # a-rows per core = 2048
K = 30             # split-precision contraction rows
KPAD = 32          # row-group stride for replicas
P = 128            # partitions
MM_N = 512         # matmul free dim per PSUM bank
GRP = 2048         # psum group = 4 matmuls of 512 (4 banks)

# column layout of the fused input tensor: [Wa shard | Rb]
OFF_WA = 0
OFF_RB = NS
TOT_COLS = NS + N

NEG_INF = -3.0e38

_CACHE = {}


def _build_nc():
    from contextlib import ExitStack

    import concourse.bacc as bacc
    import concourse.bass_isa as bass_isa
    import concourse.mybir as mybir
    import concourse.tile as tile

    bf16 = mybir.dt.bfloat16
    f32 = mybir.dt.float32
    AX = mybir.AxisListType.X
    MAX = mybir.AluOpType.max

    nc = bacc.Bacc()
    aug = nc.dram_tensor("aug", [P, TOT_COLS], bf16, kind="ExternalInput")
    # row_out[p, n] = max_j -d2(a[n*128+p], b[j])
    # col_out[mg, c] = max over this core's a of -d2(a_i, b[mg*2048+c])
    # (the last m-group is reduced via PE transposes instead of the GPSIMD
    # partition reduce so it doesn't trail the kernel; its layout is
    # col7_out[p, t] = col max for j = 7*2048 + t*128 + p)
    row_out = nc.dram_tensor("row_out", [P, NS // P], f32, kind="ExternalOutput")
    col_out = nc.dram_tensor(
        "col_out", [N // GRP - 1, GRP], f32, kind="ExternalOutput"
    )
    col7_out = nc.dram_tensor("col7_out", [P, GRP // P], f32, kind="ExternalOutput")

    n_chunks = NS // P              # 16
    m_groups = N // GRP             # 8

    with tile.TileContext(nc) as tc, ExitStack() as ctx:
        sb = ctx.enter_context(tc.tile_pool(name="sb", bufs=1))
        ps = ctx.enter_context(tc.tile_pool(name="ps", bufs=2, space="PSUM"))
        cnvp = ctx.enter_context(tc.tile_pool(name="cnvp", bufs=4))
        runp = ctx.enter_context(tc.tile_pool(name="runp", bufs=2))
        colp = ctx.enter_context(tc.tile_pool(name="colp", bufs=6))
        prp = ctx.enter_context(tc.tile_pool(name="prp", bufs=2))
        outp = ctx.enter_context(tc.tile_pool(name="outp", bufs=1))

        # Input DMA parallelized across the two HWDGE-capable engines; the
        # head slice (Wa + first Rb group) is partition-split so the first
        # matmul can start in ~1/4 the time.
        aug_sb = sb.tile([P, TOT_COLS], bf16)
        c1 = OFF_RB + GRP
        qengines = [nc.sync, nc.scalar, nc.sync, nc.scalar]
        for qi, eng in enumerate(qengines):
            eng.dma_start(
                out=aug_sb[qi * 32:(qi + 1) * 32, 0:c1],
                in_=aug[qi * 32:(qi + 1) * 32, 0:c1],
            )
        # bulk input rides the scalar-engine HWDGE queue (measured much
        # faster than the sync queue, which also carries the outputs)
        half = OFF_RB + GRP + (TOT_COLS - c1) // 2
        nc.scalar.dma_start(out=aug_sb[:, c1:half], in_=aug[:, c1:half])
        nc.scalar.dma_start(out=aug_sb[:, half:], in_=aug[:, half:])

        # Per-a-chunk running row maxes, alive across the whole kernel.
        # Initialized by copying the first m-group's tile (no memset needed).
        run_rows = sb.tile([P, n_chunks, GRP], bf16)

        row_acc = outp.tile([P, NS // P], f32)
        col7_acc = outp.tile([P, GRP // P], f32)

        from concourse.masks import make_identity

        ident = sb.tile([P, P], bf16)
        make_identity(nc, ident[:, :])

        def packed_group(pt, w_off, r_off):
            """4 concurrent matmuls (row groups g=0..3) filling pt[128,2048].
            Row group g handles the g-th 512-column sub-slice."""
            for g in range(4):
                bp = KPAD * g
                nc.tensor.matmul(
                    pt[:, g * MM_N:(g + 1) * MM_N],
                    aug_sb[bp:bp + K, w_off:w_off + P],
                    aug_sb[bp:bp + K, r_off + g * MM_N:r_off + (g + 1) * MM_N],
                    start=True,
                    stop=True,
                    tile_position=(bp, 0),
                )

        def fold_row(n):
            """run_rows[:, n, :] -> max over free axis -> row_acc[:, n]."""
            f1 = runp.tile([P, 1024], bf16, tag="f1")
            nc.vector.tensor_tensor(
                out=f1[:, :], in0=run_rows[:, n, 0:1024],
                in1=run_rows[:, n, 1024:2048], op=MAX,
            )
            f2 = runp.tile([P, 512], bf16, tag="f2")
            nc.vector.tensor_tensor(
                out=f2[:, :], in0=f1[:, 0:512], in1=f1[:, 512:1024], op=MAX,
            )
            nc.vector.tensor_reduce(row_acc[:, n:n + 1], f2[:, :], axis=AX, op=MAX)

        for mg in range(m_groups):
            run_col = colp.tile([P, GRP], bf16, tag="run_col")
            for n in range(n_chunks):
                pt = ps.tile([P, GRP], f32, tag="pt")
                packed_group(pt, OFF_WA + n * P, OFF_RB + mg * GRP)
                t = cnvp.tile([P, GRP], bf16, tag="cnv")
                nc.scalar.copy(t[:, :], pt[:, :])
                if mg == 0:
                    nc.vector.tensor_copy(run_rows[:, n, :], t[:, :])
                else:
                    nc.vector.tensor_tensor(
                        out=run_rows[:, n, :], in0=run_rows[:, n, :],
                        in1=t[:, :], op=MAX,
                    )
                if n == 0:
                    nc.vector.tensor_copy(run_col[:, :], t[:, :])
                else:
                    nc.vector.tensor_tensor(
                        out=run_col[:, :], in0=run_col[:, :], in1=t[:, :], op=MAX,
                    )
                if mg == m_groups - 1:
                    fold_row(n)
            if mg < m_groups - 1:
                pr = prp.tile([P, GRP], f32, tag="pr")
                nc.gpsimd.partition_all_reduce(
                    pr[:, :], run_col[:, :], channels=P,
                    reduce_op=bass_isa.ReduceOp.max,
                )
                nc.sync.dma_start(out=col_out[mg:mg + 1, :], in_=pr[0:1, :])
            else:
                # Tail m-group: partition-reduce via PE transposes + DVE
                # (PE/DVE are idle by now; GPSIMD would trail the kernel).
                for tb in range(GRP // P):
                    tp = ps.tile([P, P], bf16, tag="pt")
                    nc.tensor.transpose(
                        tp[:, :], run_col[:, tb * P:(tb + 1) * P], ident[:, :]
                    )
                    nc.vector.tensor_reduce(
                        col7_acc[:, tb:tb + 1], tp[:, :], axis=AX, op=MAX
                    )
                nc.sync.dma_start(out=col7_out[:, :], in_=col7_acc[:, :])
        nc.sync.dma_start(out=row_out[:, :], in_=row_acc[:, :])

    nc.compile()
    return nc


def _get_nc():
    if "nc" not in _CACHE:
        _CACHE["nc"] = _build_nc()
    return _CACHE["nc"]


def _install_ntff_hook():
    """The agent image's `antenv` lacks `axon_hooks`; provide it so
    run_bass_kernel_spmd(trace=True) can profile via the axon PJRT .so."""
    import sys

    if "antenv.axon_hooks" in sys.modules:
        return
    try:
        import contextlib
        import ctypes
        import types

        so_path = "/opt/axon/libaxon_pjrt.so"
        lib = ctypes.CDLL(so_path)
        if not hasattr(lib, "axon_start_nrt_profile"):
            return
        lib.axon_start_nrt_profile.argtypes = [
            ctypes.POINTER(ctypes.c_int64),
            ctypes.c_size_t,
        ]
        lib.axon_start_nrt_profile.restype = ctypes.c_int64
        lib.axon_stop_nrt_profile.argtypes = [ctypes.c_char_p]
        lib.axon_stop_nrt_profile.restype = ctypes.c_int64

        @contextlib.contextmanager
        def _hook(output_dir, device_ids):
            import jax

            jax.devices()
            if device_ids:
                ids = (ctypes.c_int64 * len(device_ids))(*device_ids)
                rc = lib.axon_start_nrt_profile(ids, len(device_ids))
            else:
                rc = lib.axon_start_nrt_profile(None, 0)
            if rc != 0:
                raise RuntimeError(f"axon_start_nrt_profile rc={rc}")
            try:
                yield
            finally:
                n = lib.axon_stop_nrt_profile(str(output_dir).encode())
                if n < 0:
                    raise RuntimeError(f"axon_stop_nrt_profile rc={n}")

        mod = types.ModuleType("antenv.axon_hooks")
        mod.get_axon_ntff_profile_hook = lambda: _hook
        mod.set_axon_ntff_profile_hook = lambda h: None
        sys.modules["antenv.axon_hooks"] = mod
    except Exception:
        pass


def _run(in_maps, trace=False):
    from concourse.bass_utils import run_bass_kernel_spmd

    if trace:
        _install_ntff_hook()
    nc = _get_nc()
    res = run_bass_kernel_spmd(
        nc, in_maps, core_ids=list(range(NCORES)), trace=trace
    )
    _CACHE["last_exec_ns"] = res.exec_time_ns
    _CACHE["last_trace"] = res.instructions_and_trace
    return res.results


def _split3(x):
    """fp32 -> three bf16 pieces (returned as fp32 for further math)."""
    import ml_dtypes

    h = x.astype(ml_dtypes.bfloat16).astype(np.float32)
    r = x - h
    m = r.astype(ml_dtypes.bfloat16).astype(np.float32)
    l = (r - m).astype(np.float32)
    return h, m, l


# piece-pair schedule per coordinate: indices into (h, m, l)
_PAIRS = [(0, 0), (0, 1), (1, 0), (0, 2), (2, 0), (1, 1), (1, 2), (2, 1)]


def _build_wr(Pts, Qts, P2, Q2):
    """W from the stationary set, R from the streaming set, such that
    W[:, i] . R[:, j] = -d2(P_i, Q_j)  (negated for max-reductions)."""
    W = np.zeros((K, Pts.shape[0]), np.float32)
    R = np.zeros((K, Qts.shape[0]), np.float32)
    k = 0
    for d in range(D):
        u = _split3(2.0 * Pts[:, d])       # +2 a_d  (negated -2 a.b term)
        v = _split3(Qts[:, d])
        for wp, rp in _PAIRS:
            W[k] = u[wp]
            R[k] = v[rp]
            k += 1
    q2p = _split3(Q2)
    for t in range(3):
        W[k] = -1.0
        R[k] = q2p[t]
        k += 1
    p2p = _split3(P2)
    for t in range(3):
        W[k] = -p2p[t]
        R[k] = 1.0
        k += 1
    assert k == K
    return W, R


def kernel(a, b):
    import ml_dtypes
    import os

    a = np.ascontiguousarray(np.asarray(a, dtype=np.float32))
    b = np.ascontiguousarray(np.asarray(b, dtype=np.float32))
    assert a.shape == (N, D) and b.shape == (N, D), (a.shape, b.shape)

    a2 = np.sum(a.astype(np.float64) * a, axis=1).astype(np.float32)
    b2 = np.sum(b.astype(np.float64) * b, axis=1).astype(np.float32)

    Wa, Rb = _build_wr(a, b, a2, b2)

    trace = bool(int(os.environ.get("CHAMFER_TRACE", "0")))
    in_maps = []
    for r in range(NCORES):
        row = np.zeros((KPAD, TOT_COLS), np.float32)
        row[:K, OFF_WA:OFF_WA + NS] = Wa[:, r * NS:(r + 1) * NS]
        row[:K, OFF_RB:OFF_RB + N] = Rb
        buf = np.tile(row, (4, 1))          # replicas at partitions 0/32/64/96
        in_maps.append({"aug": buf.astype(ml_dtypes.bfloat16)})
    results = _run(in_maps, trace=trace)

    # row_out[p, n] -> row index i = n*128 + p ; shards in core order
    rows = np.concatenate(
        [-results[r]["row_out"].T.reshape(-1) for r in range(NCORES)]
    )
    # col partials (negated maxes): global min = -max over cores.
    # first 7 m-groups from col_out [7,2048]; last from col7_out [128,16]
    # where j = 7*2048 + t*128 + p.
    def core_cols(r):
        c = np.empty(N, np.float32)
        c[0:7 * GRP] = results[r]["col_out"].reshape(-1)
        c[7 * GRP:] = results[r]["col7_out"].T.reshape(-1)
        return c

    cols = -np.max(np.stack([core_cols(r) for r in range(NCORES)]), axis=0)
    mins_sq = np.concatenate([rows, cols])
    dist = np.sqrt(np.maximum(mins_sq, 0.0))
    return np.asarray(np.mean(dist), dtype=np.float32)


# revision 35
# speedup vs baseline: 1.2684x; 1.0017x over previous
"""Chamfer distance kernel for Trainium2 (8 NeuronCores, SPMD).

Math: for point sets a[16384,3], b[16384,3],
  d2(i,j) = |a_i|^2 + |b_j|^2 - 2 a_i.b_j
encoded as an augmented inner product so the TensorEngine emits (negated)
squared distances directly; every reduction is then a MAX of -d2 (the
GPSIMD partition reduce only supports max, and min/max are symmetric).

fp32 matmuls on TRN2 are ~5x slower than bf16 (hi/lo double pass).  Each
fp32 operand is instead split into three bf16 pieces (value = h + m + l)
and the piece-products needed for ~fp32 accuracy are laid out along the
contraction axis (only l*l dropped): 24 coordinate rows + 3 |b|^2 rows +
3 |a|^2 rows = K=30 <= 32, so ONE bf16 matmul per tile computes -d2 at
fp32-grade accuracy (matmul cost scales with streamed columns, not K).

K<=32 also enables 4-way row-group packing: operands are replicated at
SBUF partition offsets 0/32/64/96 and 4 matmuls run concurrently in
disjoint 32-row groups of the PE array via tile_position.

Dataflow per core (a-rows sharded, 2048 per core; b replicated):
  PE    : -d2 psum groups [128, 2048] fp32      (a-chunk x b-group)
  ACT   : copy psum -> SBUF bf16 (ScalarE is the only other engine that
          can read PSUM; DVE fp32-PSUM reads are capped at 1 elem/cycle)
  DVE   : per group, TWO bf16 tensor_tensor max ops at the 2x packed rate:
            run_row[n]  = max(run_row[n],  t)   (a->b direction)
            run_col[mg] = max(run_col[mg], t)   (b->a direction, partial)
  DVE   : fold run_row[n] along free axis -> per-a-point max
  GPSIMD: partition_all_reduce(max) folds run_col across partitions
          (the only engine that can reduce the partition axis; it is
          otherwise idle)
Loop order is m-group outer / a-chunk inner so each run_col finalizes
early and its partition reduce overlaps the next group's stream.

Host: negate, sqrt, combine the 8 cores' partial b->a vectors with an
elementwise min, mean.  (min/sqrt commute; host work is 8*18k floats.)
"""

import numpy as np

N = 16384          # points in each set
D = 3
NCORES = 8
NS = N // NCORES   # a-rows per core = 2048
K = 30             # split-precision contraction rows
KPAD = 32          # row-group stride for replicas
P = 128            # partitions
MM_N = 512         # matmul free dim per PSUM bank
GRP = 2048         # psum group = 4 matmuls of 512 (4 banks)

# column layout of the fused input tensor: [Wa shard | Rb]
OFF_WA = 0
OFF_RB = NS
TOT_COLS = NS + N

NEG_INF = -3.0e38

_CACHE = {}


def _build_nc():
    from contextlib import ExitStack

    import concourse.bacc as bacc
    import concourse.bass_isa as bass_isa
    import concourse.mybir as mybir
    import concourse.tile as tile

    bf16 = mybir.dt.bfloat16
    f32 = mybir.dt.float32
    AX = mybir.AxisListType.X
    MAX = mybir.AluOpType.max

    nc = bacc.Bacc()
    aug = nc.dram_tensor("aug", [P, TOT_COLS], bf16, kind="ExternalInput")
    # row_out[p, n] = max_j -d2(a[n*128+p], b[j])
    # col_out[mg, c] = max over this core's a of -d2(a_i, b[mg*2048+c])
    # (the last m-group is reduced via PE transposes instead of the GPSIMD
    # partition reduce so it doesn't trail the kernel; its layout is
    # col7_out[p, t] = col max for j = 7*2048 + t*128 + p)
    row_out = nc.dram_tensor("row_out", [P, NS // P], f32, kind="ExternalOutput")
    col_out = nc.dram_tensor(
        "col_out", [N // GRP - 1, GRP], f32, kind="ExternalOutput"
    )
    col7_out = nc.dram_tensor("col7_out", [P, GRP // P], f32, kind="ExternalOutput")

    n_chunks = NS // P              # 16
    m_groups = N // GRP             # 8

    with tile.TileContext(nc) as tc, ExitStack() as ctx:
        sb = ctx.enter_context(tc.tile_pool(name="sb", bufs=1))
        ps = ctx.enter_context(tc.tile_pool(name="ps", bufs=2, space="PSUM"))
        cnvp = ctx.enter_context(tc.tile_pool(name="cnvp", bufs=6))
        runp = ctx.enter_context(tc.tile_pool(name="runp", bufs=2))
        colp = ctx.enter_context(tc.tile_pool(name="colp", bufs=6))
        prp = ctx.enter_context(tc.tile_pool(name="prp", bufs=2))
        outp = ctx.enter_context(tc.tile_pool(name="outp", bufs=1))

        # Input DMA parallelized across the two HWDGE-capable engines; the
        # head slice (Wa + first Rb group) is partition-split so the first
        # matmul can start in ~1/4 the time.
        aug_sb = sb.tile([P, TOT_COLS], bf16)
        c1 = OFF_RB + GRP
        qengines = [nc.sync, nc.scalar, nc.sync, nc.scalar]
        for qi, eng in enumerate(qengines):
            eng.dma_start(
                out=aug_sb[qi * 32:(qi + 1) * 32, 0:c1],
                in_=aug[qi * 32:(qi + 1) * 32, 0:c1],
            )
        # bulk input rides the scalar-engine HWDGE queue (measured much
        # faster than the sync queue, which also carries the outputs)
        half = OFF_RB + GRP + (TOT_COLS - c1) // 2
        nc.scalar.dma_start(out=aug_sb[:, c1:half], in_=aug[:, c1:half])
        nc.scalar.dma_start(out=aug_sb[:, half:], in_=aug[:, half:])

        # Per-a-chunk running row maxes, alive across the whole kernel.
        # Initialized by copying the first m-group's tile (no memset needed).
        run_rows = sb.tile([P, n_chunks, GRP], bf16)

        row_acc = outp.tile([P, NS // P], f32)
        col7_acc = outp.tile([P, GRP // P], f32)

        from concourse.masks import make_identity

        ident = sb.tile([P, P], bf16)
        make_identity(nc, ident[:, :])

        def packed_group(pt, w_off, r_off):
            """4 concurrent matmuls (row groups g=0..3) filling pt[128,2048].
            Row group g handles the g-th 512-column sub-slice."""
            for g in range(4):
                bp = KPAD * g
                nc.tensor.matmul(
                    pt[:, g * MM_N:(g + 1) * MM_N],
                    aug_sb[bp:bp + K, w_off:w_off + P],
                    aug_sb[bp:bp + K, r_off + g * MM_N:r_off + (g + 1) * MM_N],
                    start=True,
                    stop=True,
                    tile_position=(bp, 0),
                )

        def fold_row(n):
            """run_rows[:, n, :] -> max over free axis -> row_acc[:, n]."""
            f1 = runp.tile([P, 1024], bf16, tag="f1")
            nc.vector.tensor_tensor(
                out=f1[:, :], in0=run_rows[:, n, 0:1024],
                in1=run_rows[:, n, 1024:2048], op=MAX,
            )
            f2 = runp.tile([P, 512], bf16, tag="f2")
            nc.vector.tensor_tensor(
                out=f2[:, :], in0=f1[:, 0:512], in1=f1[:, 512:1024], op=MAX,
            )
            nc.vector.tensor_reduce(row_acc[:, n:n + 1], f2[:, :], axis=AX, op=MAX)

        for mg in range(m_groups):
            run_col = colp.tile([P, GRP], bf16, tag="run_col")
            for n in range(n_chunks):
                pt = ps.tile([P, GRP], f32, tag="pt")
                packed_group(pt, OFF_WA + n * P, OFF_RB + mg * GRP)
                t = cnvp.tile([P, GRP], bf16, tag="cnv")
                nc.scalar.copy(t[:, :], pt[:, :])
                if mg == 0:
                    nc.vector.tensor_copy(run_rows[:, n, :], t[:, :])
                else:
                    nc.vector.tensor_tensor(
                        out=run_rows[:, n, :], in0=run_rows[:, n, :],
                        in1=t[:, :], op=MAX,
                    )
                if n == 0:
                    nc.vector.tensor_copy(run_col[:, :], t[:, :])
                else:
                    nc.vector.tensor_tensor(
                        out=run_col[:, :], in0=run_col[:, :], in1=t[:, :], op=MAX,
                    )
                if mg == m_groups - 1:
                    fold_row(n)
            if mg < m_groups - 1:
                pr = prp.tile([P, GRP], f32, tag="pr")
                nc.gpsimd.partition_all_reduce(
                    pr[:, :], run_col[:, :], channels=P,
                    reduce_op=bass_isa.ReduceOp.max,
                )
                nc.sync.dma_start(out=col_out[mg:mg + 1, :], in_=pr[0:1, :])
            else:
                # Tail m-group: partition-reduce via PE transposes + DVE
                # (PE/DVE are idle by now; GPSIMD would trail the kernel).
                for tb in range(GRP // P):
                    tp = ps.tile([P, P], bf16, tag="pt")
                    nc.tensor.transpose(
                        tp[:, :], run_col[:, tb * P:(tb + 1) * P], ident[:, :]
                    )
                    nc.vector.tensor_reduce(
                        col7_acc[:, tb:tb + 1], tp[:, :], axis=AX, op=MAX
                    )
                nc.sync.dma_start(out=col7_out[:, :], in_=col7_acc[:, :])
        nc.sync.dma_start(out=row_out[:, :], in_=row_acc[:, :])

    nc.compile()
    return nc


def _get_nc():
    if "nc" not in _CACHE:
        _CACHE["nc"] = _build_nc()
    return _CACHE["nc"]


def _install_ntff_hook():
    """The agent image's `antenv` lacks `axon_hooks`; provide it so
    run_bass_kernel_spmd(trace=True) can profile via the axon PJRT .so."""
    import sys

    if "antenv.axon_hooks" in sys.modules:
        return
    try:
        import contextlib
        import ctypes
        import types

        so_path = "/opt/axon/libaxon_pjrt.so"
        lib = ctypes.CDLL(so_path)
        if not hasattr(lib, "axon_start_nrt_profile"):
            return
        lib.axon_start_nrt_profile.argtypes = [
            ctypes.POINTER(ctypes.c_int64),
            ctypes.c_size_t,
        ]
        lib.axon_start_nrt_profile.restype = ctypes.c_int64
        lib.axon_stop_nrt_profile.argtypes = [ctypes.c_char_p]
        lib.axon_stop_nrt_profile.restype = ctypes.c_int64

        @contextlib.contextmanager
        def _hook(output_dir, device_ids):
            import jax

            jax.devices()
            if device_ids:
                ids = (ctypes.c_int64 * len(device_ids))(*device_ids)
                rc = lib.axon_start_nrt_profile(ids, len(device_ids))
            else:
                rc = lib.axon_start_nrt_profile(None, 0)
            if rc != 0:
                raise RuntimeError(f"axon_start_nrt_profile rc={rc}")
            try:
                yield
            finally:
                n = lib.axon_stop_nrt_profile(str(output_dir).encode())
                if n < 0:
                    raise RuntimeError(f"axon_stop_nrt_profile rc={n}")

        mod = types.ModuleType("antenv.axon_hooks")
        mod.get_axon_ntff_profile_hook = lambda: _hook
        mod.set_axon_ntff_profile_hook = lambda h: None
        sys.modules["antenv.axon_hooks"] = mod
    except Exception:
        pass


def _run(in_maps, trace=False):
    from concourse.bass_utils import run_bass_kernel_spmd

    if trace:
        _install_ntff_hook()
    nc = _get_nc()
    res = run_bass_kernel_spmd(
        nc, in_maps, core_ids=list(range(NCORES)), trace=trace
    )
    _CACHE["last_exec_ns"] = res.exec_time_ns
    _CACHE["last_trace"] = res.instructions_and_trace
    return res.results


def _split3(x):
    """fp32 -> three bf16 pieces (returned as fp32 for further math)."""
    import ml_dtypes

    h = x.astype(ml_dtypes.bfloat16).astype(np.float32)
    r = x - h
    m = r.astype(ml_dtypes.bfloat16).astype(np.float32)
    l = (r - m).astype(np.float32)
    return h, m, l


# piece-pair schedule per coordinate: indices into (h, m, l)
_PAIRS = [(0, 0), (0, 1), (1, 0), (0, 2), (2, 0), (1, 1), (1, 2), (2, 1)]


def _build_wr(Pts, Qts, P2, Q2):
    """W from the stationary set, R from the streaming set, such that
    W[:, i] . R[:, j] = -d2(P_i, Q_j)  (negated for max-reductions)."""
    W = np.zeros((K, Pts.shape[0]), np.float32)
    R = np.zeros((K, Qts.shape[0]), np.float32)
    k = 0
    for d in range(D):
        u = _split3(2.0 * Pts[:, d])       # +2 a_d  (negated -2 a.b term)
        v = _split3(Qts[:, d])
        for wp, rp in _PAIRS:
            W[k] = u[wp]
            R[k] = v[rp]
            k += 1
    q2p = _split3(Q2)
    for t in range(3):
        W[k] = -1.0
        R[k] = q2p[t]
        k += 1
    p2p = _split3(P2)
    for t in range(3):
        W[k] = -p2p[t]
        R[k] = 1.0
        k += 1
    assert k == K
    return W, R


def kernel(a, b):
    import ml_dtypes
    import os

    a = np.ascontiguousarray(np.asarray(a, dtype=np.float32))
    b = np.ascontiguousarray(np.asarray(b, dtype=np.float32))
    assert a.shape == (N, D) and b.shape == (N, D), (a.shape, b.shape)

    a2 = np.sum(a.astype(np.float64) * a, axis=1).astype(np.float32)
    b2 = np.sum(b.astype(np.float64) * b, axis=1).astype(np.float32)

    Wa, Rb = _build_wr(a, b, a2, b2)

    trace = bool(int(os.environ.get("CHAMFER_TRACE", "0")))
    in_maps = []
    for r in range(NCORES):
        row = np.zeros((KPAD, TOT_COLS), np.float32)
        row[:K, OFF_WA:OFF_WA + NS] = Wa[:, r * NS:(r + 1) * NS]
        row[:K, OFF_RB:OFF_RB + N] = Rb
        buf = np.tile(row, (4, 1))          # replicas at partitions 0/32/64/96
        in_maps.append({"aug": buf.astype(ml_dtypes.bfloat16)})
    results = _run(in_maps, trace=trace)

    # row_out[p, n] -> row index i = n*128 + p ; shards in core order
    rows = np.concatenate(
        [-results[r]["row_out"].T.reshape(-1) for r in range(NCORES)]
    )
    # col partials (negated maxes): global min = -max over cores.
    # first 7 m-groups from col_out [7,2048]; last from col7_out [128,16]
    # where j = 7*2048 + t*128 + p.
    def core_cols(r):
        c = np.empty(N, np.float32)
        c[0:7 * GRP] = results[r]["col_out"].reshape(-1)
        c[7 * GRP:] = results[r]["col7_out"].T.reshape(-1)
        return c

    cols = -np.max(np.stack([core_cols(r) for r in range(NCORES)]), axis=0)
    mins_sq = np.concatenate([rows, cols])
    dist = np.sqrt(np.maximum(mins_sq, 0.0))
    return np.asarray(np.mean(dist), dtype=np.float32)
